# revision 1
# baseline (speedup 1.0000x reference)
"""Trainium2 Bass kernel for nn_ExpressionModel (dense DiT-style transformer block).

Sharding: 8 cores = 2 (batch) x 4 (sequence chunks of 512 tokens).
Each core computes the full block for its 512 query tokens; K/V projections
for the full 2048-token batch are duplicated across the 4 cores of a batch
(no collectives needed).

Everything on device runs in a channels-on-partitions ("transposed") layout:
the residual stream is xT (C=1024 rows over 8 partition-tiles, L columns).
All matmuls keep weights stationary (lhsT) and stream activations.
Matmul inputs are bf16; accumulation and the residual stream are fp32.
"""

import numpy as np
import ml_dtypes

import concourse.bass as bass
import concourse.tile as tile
from concourse import bacc, mybir
from concourse.bass_utils import run_bass_kernel_spmd

FP32 = mybir.dt.float32
BF16 = mybir.dt.bfloat16

STAGE_MARKS = []  # (instruction-id watermark, stage name) — profiling aid

B, L, C = 2, 2048, 1024
H, D = 16, 64
L2, TD = 512, 768
FF = 4096
EPS = 1e-6
NCORE = 8
LQ = 512            # query tokens per core
CT = C // 128       # 8 C partition-tiles
LKT = L // 128      # 16 key chunks (self)
LCH = L // 512      # 4 512-token chunks
KSC = 1.0 / 8.0     # 1/sqrt(D)


def build_bass():
    nc = bacc.Bacc("TRN2", target_bir_lowering=False, debug=False)
    STAGE_MARKS.clear()

    def mark(stage):
        STAGE_MARKS.append((nc.next_id(), stage))

    dma_rr = [0]

    def dma(out, in_):
        # round-robin between the two HW DGE queues (SP + ACT)
        dma_rr[0] ^= 1
        return nc.sync.dma_start(out=out, in_=in_)

    dram = {}

    def din(name, shape, dt):
        dram[name] = nc.dram_tensor(name, list(shape), dt, kind="ExternalInput")
        return dram[name]

    x_bf = din("x_bf", (C, L), BF16)           # x[b].T, bf16
    xq_f = din("xq_f", (C, LQ), FP32)          # own-chunk x[b].T, fp32 residual
    aud = din("aud", (TD, L2), BF16)           # audio_context[b].T
    tmodT = din("tmodT", (128, CT), FP32)      # t_mod[b] as columns
    cs4 = din("cs4", (128, L), BF16)   # rows [c;c;c;c] blocks, all L
    sc4 = din("sc4", (128, L), BF16)   # rows [-s;+s;-s;+s] blocks
    adabT = din("adabT", (128, 48), FP32)      # adaLN_b as columns
    n1w = din("n1w", (128, CT), FP32)
    n2w = din("n2w", (128, CT), FP32)
    n3w = din("n3w", (128, CT), FP32)
    wq_d = din("wq_d", (C, C), BF16)           # W_qkv q block, rope-permuted
    wk_d = din("wk_d", (C, C), BF16)           # W_qkv k block, rope-permuted
    wv_d = din("wv_d", (C, C), BF16)           # W_qkv v block
    wsa_d = din("wsa_d", (C, C), BF16)
    wqc_d = din("wqc_d", (C, C), BF16)         # cross-attn W_q
    wkv_d = din("wkv_d", (TD, 2 * C), BF16)
    wca_d = din("wca_d", (C, C), BF16)
    wg_d = din("wg_d", (8, 8, 128, 512), BF16)  # W_gate (mg, k, 128, 512)
    wu_d = din("wu_d", (8, 8, 128, 512), BF16)
    wd_d = din("wd_d", (FF, C), BF16)
    wada_d = din("wada_d", (12, 8, 128, 512), BF16)  # adaLN_W (n, k, 128, 512)

    outT = nc.dram_tensor("outT", [C, LQ], FP32, kind="ExternalOutput")

    with tile.TileContext(nc) as tc:
        with (
            tc.tile_pool(name="pp", bufs=1) as pp,              # persistent
            tc.tile_pool(name="pw", bufs=1) as pw,              # streamed weights
            tc.tile_pool(name="ps", bufs=1, space="PSUM") as ps,
        ):
            # ---- constants ----
            c_cs4 = pp.tile([128, L], BF16, tag="c_cs4")
            c_sc4 = pp.tile([128, L], BF16, tag="c_sc4")
            c_tmod = pp.tile([128, CT], FP32, tag="c_tmod")
            c_adab = pp.tile([128, 48], FP32, tag="c_adab")
            c_n1 = pp.tile([128, CT], FP32, tag="c_n1")
            c_n2 = pp.tile([128, CT], FP32, tag="c_n2")
            c_n3 = pp.tile([128, CT], FP32, tag="c_n3")
            dma(out=c_cs4, in_=cs4[:, :])
            dma(out=c_sc4, in_=sc4[:, :])
            dma(out=c_tmod, in_=tmodT[:, :])
            dma(out=c_adab, in_=adabT[:, :])
            dma(out=c_n1, in_=n1w[:, :])
            dma(out=c_n2, in_=n2w[:, :])
            dma(out=c_n3, in_=n3w[:, :])

            ones_col = pp.tile([128, 1], BF16, tag="ones_col")
            nc.gpsimd.memset(ones_col, 1.0)
            ones_row = pp.tile([1, 128], BF16, tag="ones_row")
            nc.gpsimd.memset(ones_row, 1.0)
            one_1 = pp.tile([1, 1], FP32, tag="one_1")
            nc.gpsimd.memset(one_1, 1.0)
            eps_c = pp.tile([1, 1], FP32, tag="eps_c")
            nc.gpsimd.memset(eps_c, EPS)

            # ---- residual (own chunk), fp32 ----
            xres = [pp.tile([128, LQ], FP32, tag=f"xres{k}", name=f"xres{k}") for k in range(CT)]
            for k in range(CT):
                dma(out=xres[k], in_=xq_f[k * 128:(k + 1) * 128, :])

            mark("adaLN")
            # =========== adaLN modulation ===========
            silu_bf = pp.tile([128, CT], BF16, tag="silu_bf")
            sg_t = pp.tile([128, CT], FP32, tag="sg_t")
            nc.scalar.activation(out=sg_t, in_=c_tmod,
                                 func=mybir.ActivationFunctionType.Sigmoid)
            nc.vector.tensor_mul(silu_bf, sg_t, c_tmod)
            modsT = pp.tile([128, 48], FP32, tag="modsT")
            w1eff = pp.tile([128, CT], FP32, tag="w1eff")
            w3eff = pp.tile([128, CT], FP32, tag="w3eff")
            # order: scale_sa (n=2,3), shift_sa (n=0,1) first — they gate
            # norm1; everything else can trail.
            for n in range(12):
                pm = ps.tile([1, 512], FP32, tag="pB", bufs=3, name=f"pm{n}")
                for k in range(8):
                    wt = pw.tile([128, 512], BF16, tag="bigw", bufs=8, name=f"wada{n}_{k}")
                    dma(out=wt, in_=wada_d[n, k])
                    nc.tensor.matmul(pm, silu_bf[:, k:k + 1], wt,
                                     start=(k == 0), stop=(k == 7))
                mrow = pp.tile([1, 512], FP32, tag="mrow", bufs=2, name=f"mrow{n}")
                nc.scalar.copy(out=mrow, in_=pm)
                # transpose row chunk -> modsT columns: modsT[p, j] = mods[j*128+p]
                for jj in range(4):
                    j = n * 4 + jj
                    pt = ps.tile([128, 1], FP32, tag="pB", bufs=3, name=f"pt{j}")
                    nc.tensor.matmul(pt, mrow[:, jj * 128:(jj + 1) * 128], one_1,
                                     start=True, stop=True)
                    nc.scalar.activation(out=modsT[:, j:j + 1], in_=pt,
                                         func=mybir.ActivationFunctionType.Identity,
                                         bias=c_adab[:, j:j + 1])
                if n == 3:
                    # w1eff = n1w * (1 + scale_sa): cols 8..16 now ready
                    nc.vector.tensor_scalar(out=w1eff, in0=modsT[:, 8:16],
                                            scalar1=1.0, scalar2=None,
                                            op0=mybir.AluOpType.add)
                    nc.vector.tensor_mul(w1eff, w1eff, c_n1)
                if n == 9:
                    nc.vector.tensor_scalar(out=w3eff, in0=modsT[:, 32:40],
                                            scalar1=1.0, scalar2=None,
                                            op0=mybir.AluOpType.add)
                    nc.vector.tensor_mul(w3eff, w3eff, c_n3)

            def sh_sa(k):
                return modsT[:, 0 + k:1 + k]

            def g_sa(k):
                return modsT[:, 16 + k:17 + k]

            def sh_ml(k):
                return modsT[:, 24 + k:25 + k]

            def g_ml(k):
                return modsT[:, 40 + k:41 + k]

            # attn output accumulators (bf16, reused by cross attn)
            att = [pp.tile([128, LQ], BF16, tag=f"att{m}", name=f"att{m}") for m in range(CT)]

            with tc.tile_pool(name="pkv", bufs=1) as pkv:
                kT = [pkv.tile([128, L], BF16, tag=f"kT{m}", name=f"kT{m}") for m in range(CT)]
                vsb = [pkv.tile([128, H, D + 1], BF16, tag=f"v{t}", name=f"v{t}") for t in range(LKT)]
                qT = [pkv.tile([128, LQ], BF16, tag=f"qT{m}", name=f"qT{m}") for m in range(CT)]

                with tc.tile_pool(name="pxa", bufs=1) as pxa:
                    xsa = [pxa.tile([128, L], BF16, tag=f"xsa{k}", name=f"xsa{k}") for k in range(CT)]

                    mark("norm1")
                    # =========== norm1 + modulation over full L ===========
                    for lc in range(LCH):
                        sl = slice(lc * 512, (lc + 1) * 512)
                        pssq = ps.tile([1, 512], FP32, tag="pB", bufs=3, name=f"pssq{lc}")
                        xins = []
                        for k in range(CT):
                            xin = pxa.tile([128, 512], BF16, tag="xin", bufs=10,
                                           name=f"xin{lc}_{k}")
                            dma(out=xin, in_=x_bf[k * 128:(k + 1) * 128, sl])
                            xins.append(xin)
                            xsq = pxa.tile([128, 512], BF16, tag="xsq", bufs=2,
                                           name=f"xsq{lc}_{k}")
                            nc.vector.tensor_mul(xsq, xin, xin)
                            nc.tensor.matmul(pssq, ones_col, xsq,
                                             start=(k == 0), stop=(k == CT - 1))
                        rstd = pp.tile([1, 512], FP32, tag="rstd", bufs=2, name=f"rstd{lc}")
                        nc.scalar.activation(out=rstd, in_=pssq,
                                             func=mybir.ActivationFunctionType.Sqrt,
                                             bias=eps_c, scale=1.0 / C)
                        nc.vector.reciprocal(rstd, rstd)
                        rstd_bf = pp.tile([1, 512], BF16, tag="rstd_bf", bufs=2,
                                          name=f"rstdb{lc}")
                        nc.vector.tensor_copy(rstd_bf, rstd)
                        pb = ps.tile([128, 512], FP32, tag="pA", bufs=5, name=f"pbn1{lc}")
                        nc.tensor.matmul(pb, ones_row, rstd_bf, start=True, stop=True)
                        for k in range(CT):
                            nc.vector.scalar_tensor_tensor(
                                out=xsa[k][:, sl], in0=xins[k], scalar=w1eff[:, k:k + 1],
                                in1=pb, op0=mybir.AluOpType.mult, op1=mybir.AluOpType.mult)
                            nc.scalar.activation(
                                out=xsa[k][:, sl], in_=xsa[k][:, sl],
                                func=mybir.ActivationFunctionType.Identity,
                                bias=sh_sa(k))

                    mark("q_proj")
                    # =========== QKV projections ===========
                    # q projection (own chunk): qT[m] = (Wq.T @ xsa_own), rope
                    wqs = [pw.tile([128, C], BF16, tag="wqkv", bufs=8, name=f"wqs{k}")
                           for k in range(CT)]
                    for k in range(CT):
                        dma(out=wqs[k], in_=wq_d[k * 128:(k + 1) * 128, :])

                    def rope_apply(dst, psrc, cc):
                        # rows per tile: head pair, each [r(32) | i(32)].
                        # ACT (idle in this phase) makes a bf16 copy kb and a
                        # half-swapped copy kbs of the psum tile; RoPE is then
                        # 3 full-tile all-SBUF bf16 DVE ops (2x mode, aligned
                        # bases):  out = kb*[c] + kbs*[-s;+s]
                        cols = slice(cc * 512, (cc + 1) * 512)
                        kb = pp.tile([128, 512], BF16, tag="ropet", bufs=6, name="kb")
                        kbs = pp.tile([128, 512], BF16, tag="ropet", bufs=6, name="kbs")
                        nc.scalar.copy(out=kb, in_=psrc)
                        for hh in (0, 64):
                            r = slice(hh, hh + 32)
                            i = slice(hh + 32, hh + 64)
                            nc.scalar.copy(out=kbs[r, :], in_=psrc[i, :])
                            nc.scalar.copy(out=kbs[i, :], in_=psrc[r, :])
                        m1 = pp.tile([128, 512], BF16, tag="ropet", bufs=6, name="m1")
                        nc.vector.tensor_mul(m1, kb, c_cs4[:, cols])
                        nc.vector.tensor_mul(kbs, kbs, c_sc4[:, cols])
                        nc.vector.tensor_add(dst, m1, kbs)

                    OWN = slice(0, LQ)  # patched at runtime by chunk offset in host slicing

                    for m in range(CT):
                        pq = ps.tile([128, LQ], FP32, tag="pA", bufs=5, name=f"pq{m}")
                        for k in range(CT):
                            nc.tensor.matmul(pq, wqs[k][:, m * 128:(m + 1) * 128],
                                             xsa[k][:, OWN],
                                             start=(k == 0), stop=(k == CT - 1))
                        rope_apply(qT[m], pq, 0)

                    mark("k_proj")
                    # k projection (full L) + rope
                    wks = [pw.tile([128, C], BF16, tag="wqkv", bufs=8, name=f"wks{k}")
                           for k in range(CT)]
                    for k in range(CT):
                        dma(out=wks[k], in_=wk_d[k * 128:(k + 1) * 128, :])
                    for m in range(CT):
                        for lc in range(LCH):
                            sl = slice(lc * 512, (lc + 1) * 512)
                            pk = ps.tile([128, 512], FP32, tag="pA", bufs=5,
                                         name=f"pk{m}_{lc}")
                            for k in range(CT):
                                nc.tensor.matmul(pk, wks[k][:, m * 128:(m + 1) * 128],
                                                 xsa[k][:, sl],
                                                 start=(k == 0), stop=(k == CT - 1))
                            rope_apply(kT[m][:, sl], pk, lc)

                    mark("v_proj")
                    # v projection (full L), natural layout + ones column
                    wvs = [pw.tile([128, C], BF16, tag="wqkv", bufs=8, name=f"wvs{k}")
                           for k in range(CT)]
                    for k in range(CT):
                        dma(out=wvs[k], in_=wv_d[k * 128:(k + 1) * 128, :])
                    for t in range(LKT):
                        nc.vector.memset(vsb[t][:, :, D:D + 1], 1.0)
                        for g in range(2):
                            pv = ps.tile([128, 512], FP32, tag="pA", bufs=5,
                                         name=f"pv{t}_{g}")
                            for k in range(CT):
                                nc.tensor.matmul(
                                    pv, xsa[k][:, t * 128:(t + 1) * 128],
                                    wvs[k][:, g * 512:(g + 1) * 512],
                                    start=(k == 0), stop=(k == CT - 1))
                            nc.vector.tensor_copy(
                                vsb[t][:, g * 8:(g + 1) * 8, 0:D],
                                pv.rearrange("p (h d) -> p h d", h=8))

                mark("self_attn")
                # =========== self-attention ===========
                for h in range(H):
                    m = h // 2
                    rs = slice((h % 2) * 64, (h % 2) * 64 + 64)
                    po = ps.tile([65, LQ], FP32, tag="pB", bufs=3, name=f"po{h}")
                    for t in range(LKT):
                        psc = ps.tile([128, LQ], FP32, tag="pA", bufs=5,
                                      name=f"psc{h}_{t}")
                        nc.tensor.matmul(psc, kT[m][rs, t * 128:(t + 1) * 128],
                                         qT[m][rs, :], start=True, stop=True)
                        pexp = pp.tile([128, LQ], BF16, tag="pexp", bufs=6,
                                       name=f"pexp{h}_{t}")
                        nc.scalar.activation(out=pexp, in_=psc,
                                             func=mybir.ActivationFunctionType.Exp,
                                             scale=KSC)
                        nc.tensor.matmul(po, vsb[t][:, h, :], pexp,
                                         start=(t == 0), stop=(t == LKT - 1))
                    rec = pp.tile([1, LQ], FP32, tag="rec", bufs=2, name=f"rec{h}")
                    nc.vector.reciprocal(rec, po[64:65, :])
                    rec_bf = pp.tile([1, LQ], BF16, tag="rec_bf", bufs=2, name=f"recb{h}")
                    nc.vector.tensor_copy(rec_bf, rec)
                    pbc = ps.tile([64, LQ], FP32, tag="pA", bufs=5, name=f"pbc{h}")
                    nc.tensor.matmul(pbc, ones_row[:, 0:64], rec_bf,
                                     start=True, stop=True)
                    rb_sb = pp.tile([64, LQ], BF16, tag="rb_sb", bufs=2,
                                    name=f"rb{h}")
                    nc.scalar.copy(out=rb_sb, in_=pbc)
                    nc.vector.tensor_mul(att[m][rs, :], po[0:64, :], rb_sb)

                mark("sa_out")
                # =========== self-attn out proj + gated residual ===========
                wsas = [pw.tile([128, C], BF16, tag="wqkv", bufs=8, name=f"wsas{k}")
                        for k in range(CT)]
                for k in range(CT):
                    dma(out=wsas[k], in_=wsa_d[k * 128:(k + 1) * 128, :])
                for m in range(CT):
                    pso = ps.tile([128, LQ], FP32, tag="pA", bufs=5, name=f"pso{m}")
                    for k in range(CT):
                        nc.tensor.matmul(pso, wsas[k][:, m * 128:(m + 1) * 128],
                                         att[k], start=(k == 0), stop=(k == CT - 1))
                    nc.vector.scalar_tensor_tensor(
                        out=xres[m], in0=pso, scalar=g_sa(m), in1=xres[m],
                        op0=mybir.AluOpType.mult, op1=mybir.AluOpType.add)

            mark("cross")
            # =========== cross attention ===========
            with tc.tile_pool(name="pca", bufs=1) as pca:
                audT = [pca.tile([128, L2], BF16, tag=f"aud{k}", name=f"audT{k}") for k in range(6)]
                for k in range(6):
                    dma(out=audT[k], in_=aud[k * 128:(k + 1) * 128, :])
                wkvs = [pca.tile([128, 2 * C], BF16, tag=f"wkv{k}", name=f"wkvs{k}") for k in range(6)]
                for k in range(6):
                    dma(out=wkvs[k], in_=wkv_d[k * 128:(k + 1) * 128, :])

                # norm2 (no modulation)
                pssq = ps.tile([1, LQ], FP32, tag="pB", bufs=3, name="pssq_n2")
                xnb = [pca.tile([128, LQ], BF16, tag=f"xn{k}", name=f"xnb{k}") for k in range(CT)]
                for k in range(CT):
                    xsq = pca.tile([128, LQ], BF16, tag="xsq2", bufs=2, name=f"xsq2_{k}")
                    nc.vector.tensor_mul(xsq, xres[k], xres[k])
                    nc.tensor.matmul(pssq, ones_col, xsq,
                                     start=(k == 0), stop=(k == CT - 1))
                rstd = pp.tile([1, LQ], FP32, tag="rstd", bufs=2, name="rstd_n2")
                nc.scalar.activation(out=rstd, in_=pssq,
                                     func=mybir.ActivationFunctionType.Sqrt,
                                     bias=eps_c, scale=1.0 / C)
                nc.vector.reciprocal(rstd, rstd)
                rstd_bf = pp.tile([1, LQ], BF16, tag="rstd_bf", bufs=2, name="rstdb_n2")
                nc.vector.tensor_copy(rstd_bf, rstd)
                pb2 = ps.tile([128, LQ], FP32, tag="pA", bufs=5, name="pb_n2")
                nc.tensor.matmul(pb2, ones_row, rstd_bf, start=True, stop=True)
                for k in range(CT):
                    nc.vector.scalar_tensor_tensor(
                        out=xnb[k], in0=xres[k], scalar=c_n2[:, k:k + 1], in1=pb2,
                        op0=mybir.AluOpType.mult, op1=mybir.AluOpType.mult)

                # cross q projection
                wqcs = [pw.tile([128, C], BF16, tag="wqkv", bufs=8, name=f"wqcs{k}")
                        for k in range(CT)]
                for k in range(CT):
                    dma(out=wqcs[k], in_=wqc_d[k * 128:(k + 1) * 128, :])
                qcT = [pca.tile([128, LQ], BF16, tag=f"qc{m}", name=f"qcT{m}") for m in range(CT)]
                for m in range(CT):
                    pq = ps.tile([128, LQ], FP32, tag="pA", bufs=5, name=f"pqc{m}")
                    for k in range(CT):
                        nc.tensor.matmul(pq, wqcs[k][:, m * 128:(m + 1) * 128],
                                         xnb[k], start=(k == 0), stop=(k == CT - 1))
                    nc.scalar.copy(out=qcT[m], in_=pq)

                # cross k (transposed) and v (natural)
                kcT = [pca.tile([128, L2], BF16, tag=f"kc{m}", name=f"kcT{m}") for m in range(CT)]
                for m in range(CT):
                    pk = ps.tile([128, L2], FP32, tag="pA", bufs=5, name=f"pkc{m}")
                    for k in range(6):
                        nc.tensor.matmul(pk, wkvs[k][:, m * 128:(m + 1) * 128],
                                         audT[k], start=(k == 0), stop=(k == 5))
                    nc.scalar.copy(out=kcT[m], in_=pk)
                vcb = [pca.tile([128, H, D + 1], BF16, tag=f"vc{t}", name=f"vcb{t}") for t in range(4)]
                for t in range(4):
                    nc.vector.memset(vcb[t][:, :, D:D + 1], 1.0)
                    for g in range(2):
                        pv = ps.tile([128, 512], FP32, tag="pA", bufs=5,
                                     name=f"pvc{t}_{g}")
                        for k in range(6):
                            nc.tensor.matmul(
                                pv, audT[k][:, t * 128:(t + 1) * 128],
                                wkvs[k][:, C + g * 512:C + (g + 1) * 512],
                                start=(k == 0), stop=(k == 5))
                        nc.vector.tensor_copy(
                            vcb[t][:, g * 8:(g + 1) * 8, 0:D],
                            pv.rearrange("p (h d) -> p h d", h=8))

                mark("cross_attn")
                # attention over audio
                for h in range(H):
                    m = h // 2
                    rs = slice((h % 2) * 64, (h % 2) * 64 + 64)
                    po = ps.tile([65, LQ], FP32, tag="pB", bufs=3, name=f"poc{h}")
                    for t in range(4):
                        psc = ps.tile([128, LQ], FP32, tag="pA", bufs=5,
                                      name=f"pscc{h}_{t}")
                        nc.tensor.matmul(psc, kcT[m][rs, t * 128:(t + 1) * 128],
                                         qcT[m][rs, :], start=True, stop=True)
                        pexp = pp.tile([128, LQ], BF16, tag="pexp", bufs=6,
                                       name=f"pexpc{h}_{t}")
                        nc.scalar.activation(out=pexp, in_=psc,
                                             func=mybir.ActivationFunctionType.Exp,
                                             scale=KSC)
                        nc.tensor.matmul(po, vcb[t][:, h, :], pexp,
                                         start=(t == 0), stop=(t == 3))
                    rec = pp.tile([1, LQ], FP32, tag="rec", bufs=2, name=f"recc{h}")
                    nc.vector.reciprocal(rec, po[64:65, :])
                    rec_bf = pp.tile([1, LQ], BF16, tag="rec_bf", bufs=2,
                                     name=f"recbc{h}")
                    nc.vector.tensor_copy(rec_bf, rec)
                    pbc = ps.tile([64, LQ], FP32, tag="pA", bufs=5, name=f"pbcc{h}")
                    nc.tensor.matmul(pbc, ones_row[:, 0:64], rec_bf,
                                     start=True, stop=True)
                    rb_sb = pp.tile([64, LQ], BF16, tag="rb_sb", bufs=2,
                                    name=f"rbc{h}")
                    nc.scalar.copy(out=rb_sb, in_=pbc)
                    nc.vector.tensor_mul(att[m][rs, :], po[0:64, :], rb_sb)

                mark("ca_out")
                # cross out proj + residual (no gate)
                wcas = [pw.tile([128, C], BF16, tag="wqkv", bufs=8, name=f"wcas{k}")
                        for k in range(CT)]
                for k in range(CT):
                    dma(out=wcas[k], in_=wca_d[k * 128:(k + 1) * 128, :])
                for m in range(CT):
                    pco = ps.tile([128, LQ], FP32, tag="pA", bufs=5, name=f"pcao{m}")
                    for k in range(CT):
                        nc.tensor.matmul(pco, wcas[k][:, m * 128:(m + 1) * 128],
                                         att[k], start=(k == 0), stop=(k == CT - 1))
                    nc.vector.tensor_add(xres[m], xres[m], pco)

            mark("mlp_norm")
            # =========== SwiGLU MLP ===========
            with tc.tile_pool(name="pml", bufs=1) as pml:
                # norm3 + modulation
                pssq = ps.tile([1, LQ], FP32, tag="pB", bufs=3, name="pssq_n3")
                xmb = [pml.tile([128, LQ], BF16, tag=f"xm{k}", name=f"xmb{k}") for k in range(CT)]
                for k in range(CT):
                    xsq = pml.tile([128, LQ], BF16, tag="xsq3", bufs=2, name=f"xsq3_{k}")
                    nc.vector.tensor_mul(xsq, xres[k], xres[k])
                    nc.tensor.matmul(pssq, ones_col, xsq,
                                     start=(k == 0), stop=(k == CT - 1))
                rstd = pp.tile([1, LQ], FP32, tag="rstd", bufs=2, name="rstd_n3")
                nc.scalar.activation(out=rstd, in_=pssq,
                                     func=mybir.ActivationFunctionType.Sqrt,
                                     bias=eps_c, scale=1.0 / C)
                nc.vector.reciprocal(rstd, rstd)
                rstd_bf = pp.tile([1, LQ], BF16, tag="rstd_bf", bufs=2, name="rstdb_n3")
                nc.vector.tensor_copy(rstd_bf, rstd)
                pb3 = ps.tile([128, LQ], FP32, tag="pA", bufs=5, name="pb_n3")
                nc.tensor.matmul(pb3, ones_row, rstd_bf, start=True, stop=True)
                for k in range(CT):
                    nc.vector.scalar_tensor_tensor(
                        out=xmb[k], in0=xres[k], scalar=w3eff[:, k:k + 1], in1=pb3,
                        op0=mybir.AluOpType.mult, op1=mybir.AluOpType.mult)
                    nc.scalar.activation(out=xmb[k], in_=xmb[k],
                                         func=mybir.ActivationFunctionType.Identity,
                                         bias=sh_ml(k))

                mark("gate_up")
                hT = [pml.tile([128, LQ], BF16, tag=f"h{t}", name=f"hT{t}") for t in range(FF // 128)]
                pd_sb = [pml.tile([128, LQ], FP32, tag=f"pds{m}", name=f"pds{m}")
                         for m in range(CT)]
                for mg in range(8):
                    pg = []
                    for mi in range(4):
                        p = ps.tile([128, LQ], FP32, tag="pA", bufs=5,
                                    name=f"pg{mg}_{mi}")
                        pg.append(p)
                    for k in range(CT):
                        wt = pw.tile([128, 512], BF16, tag="bigw", bufs=8,
                                     name=f"wg{mg}_{k}")
                        dma(out=wt, in_=wg_d[mg, k])
                        for mi in range(4):
                            nc.tensor.matmul(pg[mi], wt[:, mi * 128:(mi + 1) * 128],
                                             xmb[k], start=(k == 0), stop=(k == CT - 1))
                    gbf = []
                    for mi in range(4):
                        sg = pml.tile([128, LQ], BF16, tag="sgb", bufs=4,
                                      name=f"sg{mg}_{mi}")
                        nc.scalar.activation(out=sg, in_=pg[mi],
                                             func=mybir.ActivationFunctionType.Sigmoid)
                        gb = pml.tile([128, LQ], BF16, tag="gbf", bufs=4,
                                      name=f"gbf{mg}_{mi}")
                        nc.vector.tensor_mul(gb, sg, pg[mi])
                        gbf.append(gb)
                    pu = []
                    for mi in range(4):
                        p = ps.tile([128, LQ], FP32, tag="pA", bufs=5,
                                    name=f"pu{mg}_{mi}")
                        pu.append(p)
                    for k in range(CT):
                        wt = pw.tile([128, 512], BF16, tag="bigw", bufs=8,
                                     name=f"wu{mg}_{k}")
                        dma(out=wt, in_=wu_d[mg, k])
                        for mi in range(4):
                            nc.tensor.matmul(pu[mi], wt[:, mi * 128:(mi + 1) * 128],
                                             xmb[k], start=(k == 0), stop=(k == CT - 1))
                    for mi in range(4):
                        nc.vector.tensor_mul(hT[mg * 4 + mi], gbf[mi], pu[mi])

                    # down-proj partial for the PREVIOUS mg (lag 1), so the
                    # h-tile epilogue never sits on the PE critical path
                    for dg in ([mg - 1] if mg > 0 else []) + ([7] if mg == 7 else []):
                        wds = []
                        for dk in range(4):
                            kk = dg * 4 + dk
                            wt = pml.tile([128, C], BF16, tag="wdw", bufs=8,
                                          name=f"wd{kk}")
                            dma(out=wt, in_=wd_d[kk * 128:(kk + 1) * 128, :])
                            wds.append(wt)
                        for m in range(CT):
                            pdp = ps.tile([128, LQ], FP32, tag="pB", bufs=3,
                                          name=f"pdp{dg}_{m}")
                            for dk in range(4):
                                nc.tensor.matmul(pdp,
                                                 wds[dk][:, m * 128:(m + 1) * 128],
                                                 hT[dg * 4 + dk],
                                                 start=(dk == 0), stop=(dk == 3))
                            if dg == 0:
                                nc.vector.tensor_copy(pd_sb[m], pdp)
                            else:
                                nc.vector.tensor_add(pd_sb[m], pd_sb[m], pdp)
                for m in range(CT):
                    of = pml.tile([128, LQ], FP32, tag="of", bufs=4, name=f"of{m}")
                    nc.vector.scalar_tensor_tensor(
                        out=of, in0=pd_sb[m], scalar=g_ml(m), in1=xres[m],
                        op0=mybir.AluOpType.mult, op1=mybir.AluOpType.add)
                    dma(out=outT[m * 128:(m + 1) * 128, :], in_=of)

    nc.compile()
    return nc


_ROPE_PERM = None


def _rope_perm():
    global _ROPE_PERM
    if _ROPE_PERM is None:
        p = np.zeros(C, dtype=np.int64)
        for h in range(H):
            for i in range(D // 2):
                p[h * D + i] = h * D + 2 * i
                p[h * D + D // 2 + i] = h * D + 2 * i + 1
        _ROPE_PERM = p
    return _ROPE_PERM


def _bf(a):
    return np.ascontiguousarray(a).astype(ml_dtypes.bfloat16)


def _prep_shared(W_qkv, W_sa_out, W_q, W_kv, W_ca_out, W_gate, W_up, W_down,
                 adaLN_W, adaLN_b, freqs_cos, freqs_sin, norm1_w, norm2_w, norm3_w):
    perm = _rope_perm()
    wq = W_qkv[:, 0:C][:, perm]
    wk = W_qkv[:, C:2 * C][:, perm]
    wv = W_qkv[:, 2 * C:3 * C]
    sh = {
        "wq_d": _bf(wq), "wk_d": _bf(wk), "wv_d": _bf(wv),
        "wsa_d": _bf(W_sa_out), "wqc_d": _bf(W_q), "wkv_d": _bf(W_kv),
        "wca_d": _bf(W_ca_out),
        "wg_d": _bf(W_gate.reshape(8, 128, 8, 512).transpose(2, 0, 1, 3)),
        "wu_d": _bf(W_up.reshape(8, 128, 8, 512).transpose(2, 0, 1, 3)),
        "wd_d": _bf(W_down),
        "wada_d": _bf(adaLN_W.reshape(8, 128, 12, 512).transpose(2, 0, 1, 3)),
        "adabT": np.ascontiguousarray(
            adaLN_b.reshape(48, 128).T).astype(np.float32),
        "n1w": np.ascontiguousarray(norm1_w.reshape(8, 128).T).astype(np.float32),
        "n2w": np.ascontiguousarray(norm2_w.reshape(8, 128).T).astype(np.float32),
        "n3w": np.ascontiguousarray(norm3_w.reshape(8, 128).T).astype(np.float32),
    }
    return sh


def make_in_maps(x, t_mod, audio_context, freqs_cos, freqs_sin,
                 norm1_w, norm2_w, norm3_w,
                 W_qkv, W_sa_out, W_q, W_kv, W_ca_out,
                 W_gate, W_up, W_down, adaLN_W, adaLN_b):
    sh = _prep_shared(W_qkv, W_sa_out, W_q, W_kv, W_ca_out, W_gate, W_up,
                      W_down, adaLN_W, adaLN_b, freqs_cos, freqs_sin,
                      norm1_w, norm2_w, norm3_w)
    cosT = np.ascontiguousarray(freqs_cos.T).astype(np.float32)
    sinT = np.ascontiguousarray(freqs_sin.T).astype(np.float32)

    def rep4(a):  # (32, L) -> (128, L), 4 replicated blocks
        return _bf(np.concatenate([a, a, a, a], axis=0))
    in_maps = []
    for core in range(NCORE):
        b, j = divmod(core, 4)
        # roll the token axis so this core's own 512 tokens sit at [0, LQ);
        # RoPE freqs are rolled identically so every token keeps its true
        # rotary phase, and softmax over keys is order-invariant.
        xT = np.roll(np.ascontiguousarray(x[b].T), -j * LQ, axis=1)
        m = dict(sh)
        m["x_bf"] = _bf(xT)
        m["xq_f"] = np.ascontiguousarray(xT[:, 0:LQ]).astype(np.float32)
        cr = np.roll(cosT, -j * LQ, axis=1)
        sr = np.roll(sinT, -j * LQ, axis=1)
        m["cs4"] = rep4(cr)
        m["sc4"] = _bf(np.concatenate([-sr, sr, -sr, sr], axis=0))
        m["aud"] = _bf(audio_context[b].T)
        m["tmodT"] = np.ascontiguousarray(
            t_mod[b].reshape(8, 128).T).astype(np.float32)
        in_maps.append(m)
    return in_maps


_NC_CACHE = None


def _get_nc():
    global _NC_CACHE
    if _NC_CACHE is None:
        _NC_CACHE = build_bass()
    return _NC_CACHE


def kernel(**inputs):
    # one core's program is chunk-position independent except which tokens it
    # owns; x_bf carries the full batch, xq_f/q-slicing is done host-side by
    # rotating the token axis so each core's "own" tokens sit at [0, LQ).
    nc = _get_nc()
    inputs = {k: np.asarray(v) for k, v in inputs.items()}
    in_maps = make_in_maps(**inputs)
    res = run_bass_kernel_spmd(nc, in_maps, list(range(NCORE)))
    out = np.zeros((B, L, C), np.float32)
    for core in range(NCORE):
        b, j = divmod(core, 4)
        out[b, j * LQ:(j + 1) * LQ, :] = res.results[core]["outT"].T
    return out



# revision 19
# speedup vs baseline: 1.3777x; 1.3777x over previous
"""Trainium2 Bass kernel for nn_ExpressionModel (dense DiT-style transformer block).

Sharding: 8 cores = 2 (batch) x 4 (sequence chunks of 512 tokens).
Each core computes the full block for its 512 query tokens; K/V projections
for the full 2048-token batch are duplicated across the 4 cores of a batch
(no collectives needed).

Residual stream is transposed (channels on partitions). All dense
projections run in fp8e4 with DoubleRow perf mode (two contraction rows per
PE pass); the MLP uses hi+lo fp8 splitting (T ~ T_hi + T_lo/64) for both
weights and activations on gate/up, and for weights on down, to stay inside
the error budget. Attention scores / probabilities / p@V stay bf16.
RoPE is computed from two projections (natural + host-swapped weights) so
no engine shuffles partitions: k_rope = pk*cos + pks*sin_signed — two DVE
muls (PSUM direct) + one Pool add. adaLN runs weight-stationary (1-column
matmuls, ~free on PE); only shift/scale_sa loads up front, the other 32
columns stream in during self-attention.
"""

import numpy as np
import ml_dtypes

import concourse.bass as bass
import concourse.tile as tile
from concourse import bacc, mybir
from concourse.bass_utils import run_bass_kernel_spmd

FP32 = mybir.dt.float32
BF16 = mybir.dt.bfloat16
F8 = mybir.dt.float8e4
DR = mybir.MatmulPerfMode.DoubleRow
F8NP = ml_dtypes.float8_e4m3

STAGE_MARKS = []  # (instruction-id watermark, stage name) — profiling aid

B, L, C = 2, 2048, 1024
H, D = 16, 64
L2, TD = 512, 768
FF = 4096
EPS = 1e-6
NCORE = 8
LQ = 512            # query tokens per core
CT = C // 128       # 8 C partition-tiles
KP = C // 256       # 4 DoubleRow contraction pairs over C
LKT = L // 128      # 16 key chunks (self)
LCH = L // 512      # 4 512-token chunks
KSC = 1.0 / 8.0     # 1/sqrt(D)
LOSC = 64.0         # hi/lo split scale


def build_bass():
    nc = bacc.Bacc("TRN2", target_bir_lowering=False, debug=False)
    STAGE_MARKS.clear()

    def mark(stage):
        STAGE_MARKS.append((nc.next_id(), stage))

    def dma(out, in_):
        return nc.sync.dma_start(out=out, in_=in_)

    def din(name, shape, dt):
        return nc.dram_tensor(name, list(shape), dt, kind="ExternalInput")

    # --- inputs ---
    x_bf = din("x_bf", (C, L), BF16)            # x[b].T, bf16
    xq_f = din("xq_f", (C, LQ), FP32)           # own-chunk x[b].T, fp32 residual
    aud2 = din("aud2", (128, 3, 2, L2), F8)     # audio.T fp8 DR-paired
    tmodT = din("tmodT", (128, CT), FP32)
    cs4 = din("cs4", (128, L), BF16)            # [c;c;c;c] rows
    sc4 = din("sc4", (128, L), BF16)            # [-s;+s;-s;+s] rows
    adabT = din("adabT", (128, 48), FP32)
    n1w = din("n1w", (128, CT), FP32)
    n2w = din("n2w", (128, CT), FP32)
    n3w = din("n3w", (128, CT), FP32)
    wadaA = din("wadaA", (128, CT, 2048), BF16)   # adaLN W cols j0..15
    wadaB = din("wadaB", (8, 128, CT, 512), BF16)  # adaLN W cols j16..47, 8 pieces
    wq2 = din("wq2", (128, KP, 2, C), F8)       # W_qkv q block, rope-permuted, DR-paired
    wqs2 = din("wqs2", (128, KP, 2, C), F8)     # q block, swap-permuted
    wk2 = din("wk2", (128, KP, 2, C), F8)
    wks2 = din("wks2", (128, KP, 2, C), F8)
    wv2 = din("wv2", (128, KP, 2, C), F8)
    wsa2 = din("wsa2", (128, KP, 2, C), F8)
    wqc2 = din("wqc2", (128, KP, 2, C), F8)
    wkv2 = din("wkv2", (128, 3, 2, 2 * C), F8)
    wca2 = din("wca2", (128, KP, 2, C), F8)
    wgh = din("wgh", (8, 128, KP, 2, 512), F8)  # MLP weights hi/lo fp8
    wgl = din("wgl", (8, 128, KP, 2, 512), F8)
    wuh = din("wuh", (8, 128, KP, 2, 512), F8)
    wul = din("wul", (8, 128, KP, 2, 512), F8)
    wdh = din("wdh", (CT, 128, 16, 2, 128), F8)   # W_down hi, per out C-tile
    wdl = din("wdl", (CT, 128, 16, 2, 128), F8)

    outT = nc.dram_tensor("outT", [C, LQ], FP32, kind="ExternalOutput")

    with tile.TileContext(nc) as tc:
        with (
            tc.tile_pool(name="pp", bufs=1) as pp,              # persistent
            tc.tile_pool(name="ps", bufs=1, space="PSUM") as ps,
        ):
            # ---- persistent constants ----
            c_tmod = pp.tile([128, CT], FP32, tag="c_tmod")
            c_adab = pp.tile([128, 48], FP32, tag="c_adab")
            c_n1 = pp.tile([128, CT], FP32, tag="c_n1")
            c_n2 = pp.tile([128, CT], FP32, tag="c_n2")
            c_n3 = pp.tile([128, CT], FP32, tag="c_n3")
            c_cs4 = pp.tile([128, L], BF16, tag="c_cs4")
            c_sc4 = pp.tile([128, L], BF16, tag="c_sc4")
            xres = pp.tile([128, CT, LQ], FP32, tag="xres")
            ones_col = pp.tile([128, 1], BF16, tag="ones_col")
            ones_row = pp.tile([1, 128], BF16, tag="ones_row")
            eps_c = pp.tile([1, 1], FP32, tag="eps_c")
            nc.gpsimd.memset(ones_col, 1.0)
            nc.gpsimd.memset(ones_row, 1.0)
            nc.gpsimd.memset(eps_c, EPS)
            modsT = pp.tile([128, 48], FP32, tag="modsT")
            silu_bf = pp.tile([128, CT], BF16, tag="silu_bf")
            w1eff = pp.tile([128, CT], FP32, tag="w1eff")
            w3eff = pp.tile([128, CT], FP32, tag="w3eff")
            # attn output accumulators (fp8, DR-paired; reused by cross attn)
            att2 = [pp.tile([128, 2, LQ], F8, tag=f"att{j}", name=f"att{j}")
                    for j in range(KP)]
            # cross K (transposed) / V (natural), filled during self-attn
            kcT = [pp.tile([128, L2], BF16, tag=f"kc{m}", name=f"kcT{m}")
                   for m in range(CT)]
            vcb = [pp.tile([128, H, D + 1], BF16, tag=f"vc{t}", name=f"vcb{t}")
                   for t in range(4)]

            def sh_sa(k):
                return modsT[:, 0 + k:1 + k]

            def g_sa(k):
                return modsT[:, 16 + k:17 + k]

            def sh_ml(k):
                return modsT[:, 24 + k:25 + k]

            def g_ml(k):
                return modsT[:, 40 + k:41 + k]

            with tc.tile_pool(name="pkv", bufs=1) as pkv:
                kT = [pkv.tile([128, L], BF16, tag=f"kT{m}", name=f"kT{m}")
                      for m in range(CT)]
                vsb = [pkv.tile([128, H, D + 1], BF16, tag=f"v{t}", name=f"v{t}")
                       for t in range(LKT)]
                qT = [pkv.tile([128, LQ], BF16, tag=f"qT{m}", name=f"qT{m}")
                      for m in range(CT)]

                with tc.tile_pool(name="pqw", bufs=1) as pqw:
                    # qkv weights: q/k rotate one buffer, swaps likewise
                    w_q = pqw.tile([128, KP, 2, C], F8, tag="wmain", bufs=1, name="w_q")
                    w_qs = pqw.tile([128, KP, 2, C], F8, tag="wswap", bufs=1, name="w_qs")
                    xsa2 = [pqw.tile([128, 2, L], F8, tag=f"xsa{j}", name=f"xsa{j}")
                            for j in range(KP)]
                    # streamed x (4 chunks, 2 resident) and adaLN-A (2 pieces)
                    xc = {}

                    def x_fetch(lc):
                        xc[lc] = pqw.tile([128, CT, 512], BF16, tag="xinc",
                                          bufs=2, name=f"xin{lc}")
                        dma(out=xc[lc], in_=x_bf[:, :].rearrange(
                            "(k p) l -> p k l", p=128)[:, :, lc * 512:(lc + 1) * 512])

                    wadaA_t = [pqw.tile([128, CT, 512], BF16, tag="wadaAp",
                                        bufs=1, name=f"wadaA{i}") for i in range(4)]

                    # ---- DMA issue order (SP FIFO) ----
                    dma(out=c_tmod, in_=tmodT[:, :])
                    dma(out=c_adab, in_=adabT[:, :])
                    dma(out=c_n1, in_=n1w[:, :])
                    dma(out=c_n2, in_=n2w[:, :])
                    dma(out=c_n3, in_=n3w[:, :])
                    x_fetch(0)
                    x_fetch(1)
                    for i in range(4):
                        dma(out=wadaA_t[i], in_=wadaA[:, :, i * 512:(i + 1) * 512])
                    dma(out=c_cs4, in_=cs4[:, :])
                    dma(out=c_sc4, in_=sc4[:, :])
                    dma(out=w_q, in_=wq2[:, :, :, :])
                    dma(out=w_qs, in_=wqs2[:, :, :, :])

                    mark("norm1")
                    # ---- silu(t_mod) ----
                    sg_t = pqw.tile([128, CT], FP32, tag="sg_t")
                    nc.scalar.activation(out=sg_t, in_=c_tmod,
                                         func=mybir.ActivationFunctionType.Sigmoid)
                    nc.vector.tensor_mul(silu_bf, sg_t, c_tmod)

                    pbs = {}

                    def norm1_ssq(lc):
                        pssq = ps.tile([1, 512], FP32, tag="pB", bufs=2,
                                       name=f"pssq{lc}")
                        for k in range(CT):
                            xsq = pqw.tile([128, 512], BF16, tag="xsq", bufs=2,
                                           name=f"xsq{lc}_{k}")
                            nc.vector.tensor_mul(xsq, xc[lc][:, k, :], xc[lc][:, k, :])
                            nc.tensor.matmul(pssq, ones_col, xsq,
                                             start=(k == 0), stop=(k == CT - 1))
                        rstd = pqw.tile([1, 512], FP32, tag="rstd", bufs=1,
                                        name=f"rstd{lc}")
                        nc.scalar.activation(out=rstd, in_=pssq,
                                             func=mybir.ActivationFunctionType.Sqrt,
                                             bias=eps_c, scale=1.0 / C)
                        nc.vector.reciprocal(rstd, rstd)
                        rstd_bf = pqw.tile([1, 512], BF16, tag="rstd_bf", bufs=1,
                                           name=f"rstdb{lc}")
                        nc.vector.tensor_copy(rstd_bf, rstd)
                        pb = ps.tile([128, 512], FP32, tag="pA", bufs=4,
                                     name=f"pbn1{lc}")
                        nc.tensor.matmul(pb, ones_row, rstd_bf, start=True, stop=True)
                        pbs[lc] = pb

                    def mod1(lc):
                        sl = slice(lc * 512, (lc + 1) * 512)
                        for k in range(CT):
                            dst = xsa2[k // 2][:, k % 2, sl]
                            nc.vector.scalar_tensor_tensor(
                                out=dst, in0=xc[lc][:, k, :],
                                scalar=w1eff[:, k:k + 1], in1=pbs[lc],
                                op0=mybir.AluOpType.mult,
                                op1=mybir.AluOpType.mult)
                            nc.scalar.activation(
                                out=dst, in_=dst,
                                func=mybir.ActivationFunctionType.Identity,
                                bias=sh_sa(k))

                    norm1_ssq(0)
                    norm1_ssq(1)

                    mark("adaLN")
                    # ---- adaLN part A: shift_sa + scale_sa (weight-stationary) ----
                    pmA = ps.tile([128, 16], FP32, tag="pB", bufs=2, name="pmA")
                    for j in range(16):
                        for k in range(CT):
                            nc.tensor.matmul(pmA[:, j:j + 1],
                                             wadaA_t[j // 4][:, k,
                                                             (j % 4) * 128:(j % 4 + 1) * 128],
                                             silu_bf[:, k:k + 1],
                                             start=(k == 0), stop=(k == CT - 1))
                    nc.vector.tensor_add(modsT[:, 0:16], pmA, c_adab[:, 0:16])
                    nc.vector.tensor_scalar(out=w1eff, in0=modsT[:, 8:16],
                                            scalar1=1.0, scalar2=None,
                                            op0=mybir.AluOpType.add)
                    nc.vector.tensor_mul(w1eff, w1eff, c_n1)

                    mark("mod1")
                    # ---- modulate -> xsa2 fp8 DR-paired ----
                    mod1(0)
                    x_fetch(2)
                    mod1(1)
                    x_fetch(3)
                    dma(out=xres, in_=xq_f[:, :].rearrange(
                        "(k p) l -> p k l", p=128))
                    norm1_ssq(2)
                    mod1(2)
                    norm1_ssq(3)
                    mod1(3)

                    def proj_dr(out_psum, w, m, xcols, nkp=KP):
                        for kp in range(nkp):
                            nc.tensor.matmul(out_psum,
                                             w[:, kp, :, m * 128:(m + 1) * 128],
                                             xcols(kp),
                                             start=(kp == 0), stop=(kp == nkp - 1),
                                             perf_mode=DR)

                    def rope_apply(dst, pk_, pks_, cols):
                        m1 = pp.tile([128, 512], BF16, tag="ropet", bufs=6, name="m1")
                        nc.vector.tensor_mul(m1, pk_, c_cs4[:, cols])
                        m2 = pp.tile([128, 512], BF16, tag="ropet", bufs=6, name="m2")
                        nc.vector.tensor_mul(m2, pks_, c_sc4[:, cols])
                        nc.gpsimd.tensor_add(dst, m1, m2)

                    mark("q_proj")
                    # =========== q projection (own chunk) + rope ===========
                    OWN = slice(0, LQ)
                    for m in range(CT):
                        pq = ps.tile([128, LQ], FP32, tag="pA", bufs=4, name=f"pq{m}")
                        proj_dr(pq, w_q, m, lambda kp: xsa2[kp][:, :, OWN])
                        pqs = ps.tile([128, LQ], FP32, tag="pA", bufs=4, name=f"pqs{m}")
                        proj_dr(pqs, w_qs, m, lambda kp: xsa2[kp][:, :, OWN])
                        rope_apply(qT[m], pq, pqs, OWN)

                    mark("k_proj")
                    # =========== k projection (full L) + rope ===========
                    w_k = pqw.tile([128, KP, 2, C], F8, tag="wmain", bufs=1, name="w_k")
                    w_ks = pqw.tile([128, KP, 2, C], F8, tag="wswap", bufs=1, name="w_ks")
                    w_v = pqw.tile([128, KP, 2, C], F8, tag="wmain", bufs=1, name="w_v")
                    dma(out=w_k, in_=wk2[:, :, :, :])
                    dma(out=w_ks, in_=wks2[:, :, :, :])
                    dma(out=w_v, in_=wv2[:, :, :, :])
                    for m in range(CT):
                        for lc in range(LCH):
                            sl = slice(lc * 512, (lc + 1) * 512)
                            pk = ps.tile([128, 512], FP32, tag="pA", bufs=4,
                                         name=f"pk{m}_{lc}")
                            proj_dr(pk, w_k, m, lambda kp: xsa2[kp][:, :, sl])
                            pks = ps.tile([128, 512], FP32, tag="pA", bufs=4,
                                          name=f"pks{m}_{lc}")
                            proj_dr(pks, w_ks, m, lambda kp: xsa2[kp][:, :, sl])
                            rope_apply(kT[m][:, sl], pk, pks, sl)

                    mark("v_proj")
                    # =========== v projection (full L), natural + ones col ===========
                    for t in range(LKT):
                        nc.vector.memset(vsb[t][:, :, D:D + 1], 1.0)
                        for g in range(2):
                            pv = ps.tile([128, 512], FP32, tag="pA", bufs=4,
                                         name=f"pv{t}_{g}")
                            for kp in range(KP):
                                nc.tensor.matmul(
                                    pv, xsa2[kp][:, :, t * 128:(t + 1) * 128],
                                    w_v[:, kp, :, g * 512:(g + 1) * 512],
                                    start=(kp == 0), stop=(kp == KP - 1),
                                    perf_mode=DR)
                            nc.vector.tensor_copy(
                                vsb[t][:, g * 8:(g + 1) * 8, 0:D],
                                pv.rearrange("p (h d) -> p h d", h=8))

                # pqw closed: qkv weights + xsa2 freed
                # weights/data needed during + after self-attn
                pat_cm = tc.tile_pool(name="pat", bufs=1)
                pat = pat_cm.__enter__()
                w_sa = pat.tile([128, KP, 2, C], F8, tag="w_sa")
                w_kv = pat.tile([128, 3, 2, 2 * C], F8, tag="w_kv")
                a_t = pat.tile([128, 3, 2, L2], F8, tag="a_t")
                dma(out=w_sa, in_=wsa2[:, :, :, :])
                dma(out=w_kv, in_=wkv2[:, :, :, :])
                dma(out=a_t, in_=aud2[:, :, :, :])
                wadaB_t = {}

                def adaB_fetch(i):
                    wadaB_t[i] = pat.tile([128, CT, 512], BF16, tag="wadaB",
                                          bufs=4, name=f"wadaB{i}")
                    dma(out=wadaB_t[i], in_=wadaB[i])

                def cross_kv_piece(i):
                    # i in 0..11: 8 kc tiles then 4 vc tiles
                    if i < 8:
                        m = i
                        pkc = ps.tile([128, L2], FP32, tag="pB", bufs=2, name=f"pkc{m}")
                        for kp in range(3):
                            nc.tensor.matmul(pkc,
                                             w_kv[:, kp, :, m * 128:(m + 1) * 128],
                                             a_t[:, kp, :, :],
                                             start=(kp == 0), stop=(kp == 2),
                                             perf_mode=DR)
                        nc.vector.tensor_copy(kcT[m], pkc)
                    else:
                        t = i - 8
                        nc.vector.memset(vcb[t][:, :, D:D + 1], 1.0)
                        for g in range(2):
                            pvc = ps.tile([128, 512], FP32, tag="pB", bufs=2,
                                          name=f"pvc{t}_{g}")
                            for kp in range(3):
                                nc.tensor.matmul(
                                    pvc, a_t[:, kp, :, t * 128:(t + 1) * 128],
                                    w_kv[:, kp, :, C + g * 512:C + (g + 1) * 512],
                                    start=(kp == 0), stop=(kp == 2),
                                    perf_mode=DR)
                            nc.vector.tensor_copy(
                                vcb[t][:, g * 8:(g + 1) * 8, 0:D],
                                pvc.rearrange("p (h d) -> p h d", h=8))

                def adaB_piece(i):
                    # modsT cols 16+4i .. 20+4i
                    j0 = 16 + 4 * i
                    pmB = ps.tile([128, 4], FP32, tag="pB", bufs=2, name=f"pmB{i}")
                    for jj in range(4):
                        for k in range(CT):
                            nc.tensor.matmul(pmB[:, jj:jj + 1],
                                             wadaB_t[i][:, k, jj * 128:(jj + 1) * 128],
                                             silu_bf[:, k:k + 1],
                                             start=(k == 0), stop=(k == CT - 1))
                    nc.vector.tensor_add(modsT[:, j0:j0 + 4], pmB,
                                         c_adab[:, j0:j0 + 4])
                    if i == 5:
                        nc.vector.tensor_scalar(out=w3eff, in0=modsT[:, 32:40],
                                                scalar1=1.0, scalar2=None,
                                                op0=mybir.AluOpType.add)
                        nc.vector.tensor_mul(w3eff, w3eff, c_n3)

                mark("self_attn")
                # =========== self-attention ===========
                for h in range(H):
                    m = h // 2
                    rs = slice((h % 2) * 64, (h % 2) * 64 + 64)
                    po = ps.tile([65, LQ], FP32, tag="pC", bufs=2, name=f"po{h}")
                    for t in range(LKT):
                        psc = ps.tile([128, LQ], FP32, tag="pA", bufs=4,
                                      name=f"psc{h}_{t}")
                        nc.tensor.matmul(psc, kT[m][rs, t * 128:(t + 1) * 128],
                                         qT[m][rs, :], start=True, stop=True)
                        pexp = pp.tile([128, LQ], BF16, tag="pexp", bufs=6,
                                       name=f"pexp{h}_{t}")
                        nc.scalar.activation(out=pexp, in_=psc,
                                             func=mybir.ActivationFunctionType.Exp,
                                             scale=KSC)
                        nc.tensor.matmul(po, vsb[t][:, h, :], pexp,
                                         start=(t == 0), stop=(t == LKT - 1))
                    rec = pp.tile([1, LQ], FP32, tag="rec", bufs=2, name=f"rec{h}")
                    nc.vector.reciprocal(rec, po[64:65, :])
                    rec_bf = pp.tile([1, LQ], BF16, tag="rec_bf", bufs=2, name=f"recb{h}")
                    nc.vector.tensor_copy(rec_bf, rec)
                    pbc = ps.tile([64, LQ], FP32, tag="pA", bufs=4, name=f"pbc{h}")
                    nc.tensor.matmul(pbc, ones_row[:, 0:64], rec_bf,
                                     start=True, stop=True)
                    rb_sb = pp.tile([64, LQ], BF16, tag="rb_sb", bufs=2,
                                    name=f"rb{h}")
                    nc.vector.tensor_copy(rb_sb, pbc)
                    nc.vector.tensor_mul(att2[m // 2][rs, m % 2, :], po[0:64, :], rb_sb)
                    # fill PE idle in the Act(exp)-bound window
                    if h < 8:
                        adaB_fetch(h)
                    if 2 <= h < 14:
                        cross_kv_piece(h - 2)
                    if h >= 8:
                        adaB_piece(h - 8)

                mark("sa_out")
                # =========== self-attn out proj + gated residual ===========
                for m in range(CT):
                    pso = ps.tile([128, LQ], FP32, tag="pA", bufs=4, name=f"pso{m}")
                    proj_dr(pso, w_sa, m, lambda kp: att2[kp][:, :, :])
                    nc.vector.scalar_tensor_tensor(
                        out=xres[:, m, :], in0=pso, scalar=g_sa(m), in1=xres[:, m, :],
                        op0=mybir.AluOpType.mult, op1=mybir.AluOpType.add)
                pat_cm.__exit__(None, None, None)

            mark("cross")
            # =========== cross attention + MLP ===========
            with tc.tile_pool(name="pca", bufs=1) as pca:
                w_qc = pca.tile([128, KP, 2, C], F8, tag="w_qc")
                dma(out=w_qc, in_=wqc2[:, :, :, :])
                w_ca = pca.tile([128, KP, 2, C], F8, tag="w_ca")
                dma(out=w_ca, in_=wca2[:, :, :, :])
                # MLP gate/up weight stream (2 mg ahead)
                wgh_t, wgl_t, wuh_t, wul_t = {}, {}, {}, {}

                def gu_fetch(mg):
                    for d, src_, nm in ((wgh_t, wgh, "gh"), (wgl_t, wgl, "gl"),
                                        (wuh_t, wuh, "uh"), (wul_t, wul, "ul")):
                        d[mg] = pca.tile([128, KP, 2, 512], F8, tag="wgu", bufs=8,
                                         name=f"w{nm}{mg}")
                        dma(out=d[mg], in_=src_[mg])

                gu_fetch(0)
                gu_fetch(1)

                # norm2 (no modulation) -> xnb2 fp8 DR-paired
                pssq = ps.tile([1, LQ], FP32, tag="pB", bufs=2, name="pssq_n2")
                xnb2 = [pca.tile([128, 2, LQ], F8, tag=f"xn{j}", name=f"xnb{j}")
                        for j in range(KP)]
                for k in range(CT):
                    xsq = pca.tile([128, LQ], BF16, tag="xsq2", bufs=1, name=f"xsq2_{k}")
                    nc.vector.tensor_mul(xsq, xres[:, k, :], xres[:, k, :])
                    nc.tensor.matmul(pssq, ones_col, xsq,
                                     start=(k == 0), stop=(k == CT - 1))
                rstd = pca.tile([1, LQ], FP32, tag="rstd", bufs=1, name="rstd_n2")
                nc.scalar.activation(out=rstd, in_=pssq,
                                     func=mybir.ActivationFunctionType.Sqrt,
                                     bias=eps_c, scale=1.0 / C)
                nc.vector.reciprocal(rstd, rstd)
                rstd_bf = pca.tile([1, LQ], BF16, tag="rstd_bf", bufs=1, name="rstdb_n2")
                nc.vector.tensor_copy(rstd_bf, rstd)
                pb2 = ps.tile([128, LQ], FP32, tag="pA", bufs=4, name="pb_n2")
                nc.tensor.matmul(pb2, ones_row, rstd_bf, start=True, stop=True)
                for k in range(CT):
                    nc.vector.scalar_tensor_tensor(
                        out=xnb2[k // 2][:, k % 2, :], in0=xres[:, k, :],
                        scalar=c_n2[:, k:k + 1], in1=pb2,
                        op0=mybir.AluOpType.mult, op1=mybir.AluOpType.mult)

                def proj_dr2(out_psum, w, m, xcols, nkp=KP):
                    for kp in range(nkp):
                        nc.tensor.matmul(out_psum,
                                         w[:, kp, :, m * 128:(m + 1) * 128],
                                         xcols(kp),
                                         start=(kp == 0), stop=(kp == nkp - 1),
                                         perf_mode=DR)

                # cross q projection
                qcT = [pca.tile([128, LQ], BF16, tag=f"qc{m}", name=f"qcT{m}")
                       for m in range(CT)]
                for m in range(CT):
                    pq = ps.tile([128, LQ], FP32, tag="pA", bufs=4, name=f"pqc{m}")
                    proj_dr2(pq, w_qc, m, lambda kp: xnb2[kp][:, :, :])
                    nc.vector.tensor_copy(qcT[m], pq)

                mark("cross_attn")
                # attention over audio; stream remaining MLP weights meanwhile
                for h in range(H):
                    m = h // 2
                    rs = slice((h % 2) * 64, (h % 2) * 64 + 64)
                    po = ps.tile([65, LQ], FP32, tag="pC", bufs=2, name=f"poc{h}")
                    for t in range(4):
                        psc = ps.tile([128, LQ], FP32, tag="pA", bufs=4,
                                      name=f"pscc{h}_{t}")
                        nc.tensor.matmul(psc, kcT[m][rs, t * 128:(t + 1) * 128],
                                         qcT[m][rs, :], start=True, stop=True)
                        pexp = pp.tile([128, LQ], BF16, tag="pexp", bufs=6,
                                       name=f"pexpc{h}_{t}")
                        nc.scalar.activation(out=pexp, in_=psc,
                                             func=mybir.ActivationFunctionType.Exp,
                                             scale=KSC)
                        nc.tensor.matmul(po, vcb[t][:, h, :], pexp,
                                         start=(t == 0), stop=(t == 3))
                    rec = pp.tile([1, LQ], FP32, tag="rec", bufs=2, name=f"recc{h}")
                    nc.vector.reciprocal(rec, po[64:65, :])
                    rec_bf = pp.tile([1, LQ], BF16, tag="rec_bf", bufs=2,
                                     name=f"recbc{h}")
                    nc.vector.tensor_copy(rec_bf, rec)
                    pbc = ps.tile([64, LQ], FP32, tag="pA", bufs=4, name=f"pbcc{h}")
                    nc.tensor.matmul(pbc, ones_row[:, 0:64], rec_bf,
                                     start=True, stop=True)
                    rb_sb = pp.tile([64, LQ], BF16, tag="rb_sb", bufs=2,
                                    name=f"rbc{h}")
                    nc.vector.tensor_copy(rb_sb, pbc)
                    nc.vector.tensor_mul(att2[m // 2][rs, m % 2, :], po[0:64, :], rb_sb)
                    if h % 3 == 0 and 2 + h // 3 < 8:
                        gu_fetch(2 + h // 3)

                mark("ca_out")
                # cross out proj + residual (no gate)
                for m in range(CT):
                    pco = ps.tile([128, LQ], FP32, tag="pA", bufs=4, name=f"pcao{m}")
                    proj_dr2(pco, w_ca, m, lambda kp: att2[kp][:, :, :])
                    nc.vector.tensor_add(xres[:, m, :], xres[:, m, :], pco)

                mark("mlp_norm")
                # norm3 + modulation -> bf16, then hi/lo fp8 split
                pssq3 = ps.tile([1, LQ], FP32, tag="pB", bufs=2, name="pssq_n3")
                xmb = [pca.tile([128, LQ], BF16, tag=f"xm{k}", name=f"xmb{k}")
                       for k in range(CT)]
                xh2 = [pca.tile([128, 2, LQ], F8, tag=f"xh{j}", name=f"xh{j}")
                       for j in range(KP)]
                xl2 = [pca.tile([128, 2, LQ], F8, tag=f"xl{j}", name=f"xl{j}")
                       for j in range(KP)]
                x64 = [pca.tile([128, 2, LQ], F8, tag=f"x6{j}", name=f"x6{j}")
                       for j in range(KP)]
                for k in range(CT):
                    xsq = pca.tile([128, LQ], BF16, tag="xsq2", bufs=1, name=f"xsq3_{k}")
                    nc.vector.tensor_mul(xsq, xres[:, k, :], xres[:, k, :])
                    nc.tensor.matmul(pssq3, ones_col, xsq,
                                     start=(k == 0), stop=(k == CT - 1))
                rstd3 = pca.tile([1, LQ], FP32, tag="rstd", bufs=1, name="rstd_n3")
                nc.scalar.activation(out=rstd3, in_=pssq3,
                                     func=mybir.ActivationFunctionType.Sqrt,
                                     bias=eps_c, scale=1.0 / C)
                nc.vector.reciprocal(rstd3, rstd3)
                rstd3_bf = pca.tile([1, LQ], BF16, tag="rstd_bf", bufs=1,
                                    name="rstdb_n3")
                nc.vector.tensor_copy(rstd3_bf, rstd3)
                pb3 = ps.tile([128, LQ], FP32, tag="pA", bufs=4, name="pb_n3")
                nc.tensor.matmul(pb3, ones_row, rstd3_bf, start=True, stop=True)
                for k in range(CT):
                    nc.vector.scalar_tensor_tensor(
                        out=xmb[k], in0=xres[:, k, :], scalar=w3eff[:, k:k + 1],
                        in1=pb3,
                        op0=mybir.AluOpType.mult, op1=mybir.AluOpType.mult)
                    nc.scalar.activation(out=xmb[k], in_=xmb[k],
                                         func=mybir.ActivationFunctionType.Identity,
                                         bias=sh_ml(k))
                    hi = xh2[k // 2][:, k % 2, :]
                    lo = xl2[k // 2][:, k % 2, :]
                    nc.scalar.copy(out=hi, in_=xmb[k])
                    nc.vector.tensor_sub(lo, xmb[k], hi)
                    nc.scalar.activation(out=x64[k // 2][:, k % 2, :], in_=xmb[k],
                                         func=mybir.ActivationFunctionType.Identity,
                                         scale=1.0 / LOSC)

                mark("gate_up")
                # h2: fp8 DR-paired ffn activations
                h2 = [pca.tile([128, 2, LQ], F8, tag=f"h{t}", name=f"h2_{t}")
                      for t in range(FF // 256)]
                h64_2 = [pca.tile([128, 2, LQ], F8, tag=f"h6{t}", name=f"h64_{t}")
                         for t in range(FF // 256)]
                wdh_t, wdl_t = {}, {}

                def down_fetch(m):
                    wdh_t[m] = pca.tile([128, 16, 2, 128], F8, tag="wdw", bufs=4,
                                        name=f"wdh{m}")
                    dma(out=wdh_t[m], in_=wdh[m])
                    wdl_t[m] = pca.tile([128, 16, 2, 128], F8, tag="wdw", bufs=4,
                                        name=f"wdl{m}")
                    dma(out=wdl_t[m], in_=wdl[m])

                def dr_hilo(p1, wh, wl, mi):
                    # Xh*Wh + (X/64)*(Wl*64) + Xl*Wh, all at true scale
                    ms = slice(mi * 128, (mi + 1) * 128)
                    for kp in range(KP):
                        nc.tensor.matmul(p1, wh[:, kp, :, ms], xh2[kp][:, :, :],
                                         start=(kp == 0), stop=False, perf_mode=DR)
                    for kp in range(KP):
                        nc.tensor.matmul(p1, wl[:, kp, :, ms], x64[kp][:, :, :],
                                         start=False, stop=False, perf_mode=DR)
                    for kp in range(KP):
                        nc.tensor.matmul(p1, wh[:, kp, :, ms], xl2[kp][:, :, :],
                                         start=False, stop=(kp == KP - 1),
                                         perf_mode=DR)

                for mg in range(8):
                    if mg >= 6:
                        down_fetch(mg - 6)
                    for mi in range(4):
                        p1g = ps.tile([128, LQ], FP32, tag="pA", bufs=4,
                                      name=f"p1g{mg}_{mi}")
                        dr_hilo(p1g, wgh_t[mg], wgl_t[mg], mi)
                        sg = pca.tile([128, LQ], BF16, tag="sgb", bufs=2,
                                      name=f"sg{mg}_{mi}")
                        nc.scalar.activation(out=sg, in_=p1g,
                                             func=mybir.ActivationFunctionType.Sigmoid)
                        gbf = pca.tile([128, LQ], BF16, tag="gbf", bufs=4,
                                       name=f"gbf{mg}_{mi}")
                        nc.vector.tensor_mul(gbf, sg, p1g)
                        p1u = ps.tile([128, LQ], FP32, tag="pA", bufs=4,
                                      name=f"p1u{mg}_{mi}")
                        dr_hilo(p1u, wuh_t[mg], wul_t[mg], mi)
                        t = mg * 4 + mi
                        nc.vector.tensor_mul(h2[t // 2][:, t % 2, :], gbf, p1u)
                        h64 = h64_2[t // 2][:, t % 2, :]
                        nc.scalar.activation(
                            out=h64, in_=h2[t // 2][:, t % 2, :],
                            func=mybir.ActivationFunctionType.Identity,
                            scale=1.0 / LOSC)

                mark("down")
                # down proj: P1 = H*Wdh, P2 = H*Wdl(x64); out = (P1 + P2/64)*g + xres
                for m in range(CT):
                    if m + 2 < CT:
                        down_fetch(m + 2)
                    pd1 = ps.tile([128, LQ], FP32, tag="pA", bufs=4, name=f"pd1{m}")
                    for fp in range(16):
                        nc.tensor.matmul(pd1, wdh_t[m][:, fp, :, :],
                                         h2[fp][:, :, :],
                                         start=(fp == 0), stop=False,
                                         perf_mode=DR)
                    for fp in range(16):
                        nc.tensor.matmul(pd1, wdl_t[m][:, fp, :, :],
                                         h64_2[fp][:, :, :],
                                         start=False, stop=(fp == 15),
                                         perf_mode=DR)
                    of = pca.tile([128, LQ], FP32, tag="of", bufs=2, name=f"of{m}")
                    nc.vector.scalar_tensor_tensor(
                        out=of, in0=pd1, scalar=g_ml(m), in1=xres[:, m, :],
                        op0=mybir.AluOpType.mult, op1=mybir.AluOpType.add)
                    dma(out=outT[m * 128:(m + 1) * 128, :], in_=of)

    nc.compile()
    return nc


_ROPE_PERM = None
_SWAP_PERM = None


def _perms():
    global _ROPE_PERM, _SWAP_PERM
    if _ROPE_PERM is None:
        p = np.zeros(C, dtype=np.int64)
        s = np.zeros(C, dtype=np.int64)
        for h in range(H):
            for i in range(D // 2):
                p[h * D + i] = h * D + 2 * i               # real block
                p[h * D + D // 2 + i] = h * D + 2 * i + 1  # imag block
                s[h * D + i] = h * D + 2 * i + 1           # swapped: imag first
                s[h * D + D // 2 + i] = h * D + 2 * i
        _ROPE_PERM, _SWAP_PERM = p, s
    return _ROPE_PERM, _SWAP_PERM


def _bf(a):
    return np.ascontiguousarray(a).astype(ml_dtypes.bfloat16)


def _f8(a):
    return np.ascontiguousarray(a).astype(F8NP)


def _dr_pack(W):
    # [n_in, n_out] -> [128, n_in//256, 2, n_out]
    n_in, n_out = W.shape
    kp = n_in // 256
    return W.reshape(kp, 2, 128, n_out).transpose(2, 0, 1, 3)


def _hilo(W):
    hi = W.astype(F8NP)
    lo = ((W - hi.astype(np.float32)) * LOSC).astype(F8NP)
    return hi, lo


def _prep_shared(W_qkv, W_sa_out, W_q, W_kv, W_ca_out, W_gate, W_up, W_down,
                 adaLN_W, adaLN_b, norm1_w, norm2_w, norm3_w):
    perm, sperm = _perms()
    wq = W_qkv[:, 0:C][:, perm]
    wqs = W_qkv[:, 0:C][:, sperm]
    wk = W_qkv[:, C:2 * C][:, perm]
    wks = W_qkv[:, C:2 * C][:, sperm]
    wv = W_qkv[:, 2 * C:3 * C]

    def pack8(W):
        return _f8(_dr_pack(np.asarray(W, np.float32)))

    wgh_, wgl_ = _hilo(np.asarray(W_gate, np.float32))
    wuh_, wul_ = _hilo(np.asarray(W_up, np.float32))
    wdh_, wdl_ = _hilo(np.asarray(W_down, np.float32))

    def mlp_pack(w8):  # fp8 [C, FF] -> [8 mg][128, kp, 2, 512]
        d = _dr_pack(w8.astype(np.float32)).astype(F8NP)  # [128, 4, 2, 4096]
        return np.ascontiguousarray(d.reshape(128, KP, 2, 8, 512)
                                    .transpose(3, 0, 1, 2, 4))

    def down_pack(w8):  # fp8 [FF, C] -> [8 m][128, 16 fp, 2, 128]
        d = _dr_pack(w8.astype(np.float32)).astype(F8NP)  # [128, 16, 2, C]
        return np.ascontiguousarray(d.reshape(128, 16, 2, CT, 128)
                                    .transpose(3, 0, 1, 2, 4))

    # adaLN weight-stationary tiles: [p, k, j*128+q] = W[128k+p, 128j+q]
    wada = np.asarray(adaLN_W, np.float32).reshape(CT, 128, 48, 128)
    wadaA_h = wada[:, :, 0:16, :].transpose(1, 0, 2, 3).reshape(128, CT, 2048)
    wadaB_h = np.stack([
        wada[:, :, 16 + 4 * i:20 + 4 * i, :].transpose(1, 0, 2, 3)
        .reshape(128, CT, 512) for i in range(8)])

    sh = {
        "wq2": pack8(wq), "wqs2": pack8(wqs), "wk2": pack8(wk),
        "wks2": pack8(wks), "wv2": pack8(wv),
        "wsa2": pack8(W_sa_out), "wqc2": pack8(W_q), "wkv2": pack8(W_kv),
        "wca2": pack8(W_ca_out),
        "wgh": mlp_pack(wgh_), "wgl": mlp_pack(wgl_),
        "wuh": mlp_pack(wuh_), "wul": mlp_pack(wul_),
        "wdh": down_pack(wdh_), "wdl": down_pack(wdl_),
        "wadaA": _bf(wadaA_h), "wadaB": _bf(wadaB_h),
        "adabT": np.ascontiguousarray(
            np.asarray(adaLN_b, np.float32).reshape(48, 128).T),
        "n1w": np.ascontiguousarray(
            np.asarray(norm1_w, np.float32).reshape(8, 128).T),
        "n2w": np.ascontiguousarray(
            np.asarray(norm2_w, np.float32).reshape(8, 128).T),
        "n3w": np.ascontiguousarray(
            np.asarray(norm3_w, np.float32).reshape(8, 128).T),
    }
    return sh


def make_in_maps(x, t_mod, audio_context, freqs_cos, freqs_sin,
                 norm1_w, norm2_w, norm3_w,
                 W_qkv, W_sa_out, W_q, W_kv, W_ca_out,
                 W_gate, W_up, W_down, adaLN_W, adaLN_b):
    sh = _prep_shared(W_qkv, W_sa_out, W_q, W_kv, W_ca_out, W_gate, W_up,
                      W_down, adaLN_W, adaLN_b, norm1_w, norm2_w, norm3_w)
    cosT = np.ascontiguousarray(np.asarray(freqs_cos, np.float32).T)
    sinT = np.ascontiguousarray(np.asarray(freqs_sin, np.float32).T)

    in_maps = []
    for core in range(NCORE):
        b, j = divmod(core, 4)
        # roll the token axis so this core's own 512 tokens sit at [0, LQ)
        xT = np.roll(np.ascontiguousarray(np.asarray(x, np.float32)[b].T),
                     -j * LQ, axis=1)
        m = dict(sh)
        m["x_bf"] = _bf(xT)
        m["xq_f"] = np.ascontiguousarray(xT[:, 0:LQ])
        cr = np.roll(cosT, -j * LQ, axis=1)
        sr = np.roll(sinT, -j * LQ, axis=1)
        m["cs4"] = _bf(np.concatenate([cr, cr, cr, cr], axis=0))
        m["sc4"] = _bf(np.concatenate([-sr, sr, -sr, sr], axis=0))
        m["aud2"] = _f8(_dr_pack(
            np.ascontiguousarray(np.asarray(audio_context, np.float32)[b].T)))
        m["tmodT"] = np.ascontiguousarray(
            np.asarray(t_mod, np.float32)[b].reshape(8, 128).T)
        in_maps.append(m)
    return in_maps


_NC_CACHE = None


def _get_nc():
    global _NC_CACHE
    if _NC_CACHE is None:
        _NC_CACHE = build_bass()
    return _NC_CACHE


def kernel(**inputs):
    nc = _get_nc()
    inputs = {k: np.asarray(v) for k, v in inputs.items()}
    in_maps = make_in_maps(**inputs)
    res = run_bass_kernel_spmd(nc, in_maps, list(range(NCORE)))
    out = np.zeros((B, L, C), np.float32)
    for core in range(NCORE):
        b, j = divmod(core, 4)
        out[b, j * LQ:(j + 1) * LQ, :] = res.results[core]["outT"].T
    return out


# revision 20
# speedup vs baseline: 1.4088x; 1.0225x over previous
"""Trainium2 Bass kernel for nn_ExpressionModel (dense DiT-style transformer block).

Sharding: 8 cores = 2 (batch) x 4 (sequence chunks of 512 tokens).
Each core computes the full block for its 512 query tokens; K/V projections
for the full 2048-token batch are duplicated across the 4 cores of a batch
(no collectives needed).

Residual stream is transposed (channels on partitions). All dense
projections run in fp8e4 with DoubleRow perf mode (two contraction rows per
PE pass); the MLP uses hi+lo fp8 splitting (T ~ T_hi + T_lo/64) for both
weights and activations on gate/up, and for weights on down, to stay inside
the error budget. Attention scores / probabilities / p@V stay bf16.
RoPE is computed from two projections (natural + host-swapped weights) so
no engine shuffles partitions: k_rope = pk*cos + pks*sin_signed — two DVE
muls (PSUM direct) + one Pool add. adaLN runs weight-stationary (1-column
matmuls, ~free on PE); only shift/scale_sa loads up front, the other 32
columns stream in during self-attention.
"""

import numpy as np
import ml_dtypes

import concourse.bass as bass
import concourse.tile as tile
from concourse import bacc, mybir
from concourse.bass_utils import run_bass_kernel_spmd

FP32 = mybir.dt.float32
BF16 = mybir.dt.bfloat16
F8 = mybir.dt.float8e4
DR = mybir.MatmulPerfMode.DoubleRow
F8NP = ml_dtypes.float8_e4m3

STAGE_MARKS = []  # (instruction-id watermark, stage name) — profiling aid

B, L, C = 2, 2048, 1024
H, D = 16, 64
L2, TD = 512, 768
FF = 4096
EPS = 1e-6
NCORE = 8
LQ = 512            # query tokens per core
CT = C // 128       # 8 C partition-tiles
KP = C // 256       # 4 DoubleRow contraction pairs over C
LKT = L // 128      # 16 key chunks (self)
LCH = L // 512      # 4 512-token chunks
KSC = 1.0 / 8.0     # 1/sqrt(D)
LOSC = 64.0         # hi/lo split scale


def build_bass():
    nc = bacc.Bacc("TRN2", target_bir_lowering=False, debug=False)
    STAGE_MARKS.clear()

    def mark(stage):
        STAGE_MARKS.append((nc.next_id(), stage))

    def dma(out, in_):
        return nc.sync.dma_start(out=out, in_=in_)

    def din(name, shape, dt):
        return nc.dram_tensor(name, list(shape), dt, kind="ExternalInput")

    # --- inputs ---
    x_bf = din("x_bf", (C, L), BF16)            # x[b].T, bf16
    xq_f = din("xq_f", (C, LQ), FP32)           # own-chunk x[b].T, fp32 residual
    aud2 = din("aud2", (128, 3, 2, L2), F8)     # audio.T fp8 DR-paired
    tmodT = din("tmodT", (128, CT), FP32)
    cs4 = din("cs4", (128, L), BF16)            # [c;c;c;c] rows
    sc4 = din("sc4", (128, L), BF16)            # [-s;+s;-s;+s] rows
    adabT = din("adabT", (128, 48), FP32)
    n1w = din("n1w", (128, CT), FP32)
    n2w = din("n2w", (128, CT), FP32)
    n3w = din("n3w", (128, CT), FP32)
    wadaA = din("wadaA", (128, CT, 2048), BF16)   # adaLN W cols j0..15
    wadaB = din("wadaB", (8, 128, CT, 512), BF16)  # adaLN W cols j16..47, 8 pieces
    wq2 = din("wq2", (128, KP, 2, C), F8)       # W_qkv q block, rope-permuted, DR-paired
    wqs2 = din("wqs2", (128, KP, 2, C), F8)     # q block, swap-permuted
    wk2 = din("wk2", (128, KP, 2, C), F8)
    wks2 = din("wks2", (128, KP, 2, C), F8)
    wv2 = din("wv2", (128, KP, 2, C), F8)
    wsa2 = din("wsa2", (128, KP, 2, C), F8)
    wqc2 = din("wqc2", (128, KP, 2, C), F8)
    wkv2 = din("wkv2", (128, 3, 2, 2 * C), F8)
    wca2 = din("wca2", (128, KP, 2, C), F8)
    wgh = din("wgh", (8, 128, KP, 2, 512), F8)  # MLP weights hi/lo fp8
    wgl = din("wgl", (8, 128, KP, 2, 512), F8)
    wuh = din("wuh", (8, 128, KP, 2, 512), F8)
    wul = din("wul", (8, 128, KP, 2, 512), F8)
    wdh = din("wdh", (CT, 128, 16, 2, 128), F8)   # W_down hi, per out C-tile
    wdl = din("wdl", (CT, 128, 16, 2, 128), F8)

    outT = nc.dram_tensor("outT", [C, LQ], FP32, kind="ExternalOutput")

    with tile.TileContext(nc) as tc:
        with (
            tc.tile_pool(name="pp", bufs=1) as pp,              # persistent
            tc.tile_pool(name="ps", bufs=1, space="PSUM") as ps,
        ):
            # ---- persistent constants ----
            c_tmod = pp.tile([128, CT], FP32, tag="c_tmod")
            c_adab = pp.tile([128, 48], FP32, tag="c_adab")
            c_n1 = pp.tile([128, CT], FP32, tag="c_n1")
            c_n2 = pp.tile([128, CT], FP32, tag="c_n2")
            c_n3 = pp.tile([128, CT], FP32, tag="c_n3")
            c_cs4 = pp.tile([128, L], BF16, tag="c_cs4")
            c_sc4 = pp.tile([128, L], BF16, tag="c_sc4")
            xres = pp.tile([128, CT, LQ], FP32, tag="xres")
            ones_col = pp.tile([128, 1], BF16, tag="ones_col")
            ones_row = pp.tile([1, 128], BF16, tag="ones_row")
            eps_c = pp.tile([1, 1], FP32, tag="eps_c")
            nc.gpsimd.memset(ones_col, 1.0)
            nc.gpsimd.memset(ones_row, 1.0)
            nc.gpsimd.memset(eps_c, EPS)
            modsT = pp.tile([128, 48], FP32, tag="modsT")
            silu_bf = pp.tile([128, CT], BF16, tag="silu_bf")
            w1eff = pp.tile([128, CT], FP32, tag="w1eff")
            w3eff = pp.tile([128, CT], FP32, tag="w3eff")
            # attn output accumulators (fp8, DR-paired; reused by cross attn)
            att2 = [pp.tile([128, 2, LQ], F8, tag=f"att{j}", name=f"att{j}")
                    for j in range(KP)]
            # cross K (transposed) / V (natural), filled during self-attn
            kcT = [pp.tile([128, L2], BF16, tag=f"kc{m}", name=f"kcT{m}")
                   for m in range(CT)]
            vcb = [pp.tile([128, H, D + 1], BF16, tag=f"vc{t}", name=f"vcb{t}")
                   for t in range(4)]

            def sh_sa(k):
                return modsT[:, 0 + k:1 + k]

            def g_sa(k):
                return modsT[:, 16 + k:17 + k]

            def sh_ml(k):
                return modsT[:, 24 + k:25 + k]

            def g_ml(k):
                return modsT[:, 40 + k:41 + k]

            with tc.tile_pool(name="pkv", bufs=1) as pkv:
                kT = [pkv.tile([128, L], BF16, tag=f"kT{m}", name=f"kT{m}")
                      for m in range(CT)]
                vsb = [pkv.tile([128, H, D + 1], BF16, tag=f"v{t}", name=f"v{t}")
                       for t in range(LKT)]
                qT = [pkv.tile([128, LQ], BF16, tag=f"qT{m}", name=f"qT{m}")
                      for m in range(CT)]

                with tc.tile_pool(name="pqw", bufs=1) as pqw:
                    # qkv weights: q/k rotate one buffer, swaps likewise
                    w_q = pqw.tile([128, KP, 2, C], F8, tag="wmain", bufs=1, name="w_q")
                    w_qs = pqw.tile([128, KP, 2, C], F8, tag="wswap", bufs=1, name="w_qs")
                    xsa2 = [pqw.tile([128, 2, L], F8, tag=f"xsa{j}", name=f"xsa{j}")
                            for j in range(KP)]
                    # streamed x (4 chunks, 2 resident) and adaLN-A (2 pieces)
                    xc = {}

                    def x_fetch(lc):
                        xc[lc] = pqw.tile([128, CT, 512], BF16, tag="xinc",
                                          bufs=2, name=f"xin{lc}")
                        dma(out=xc[lc], in_=x_bf[:, :].rearrange(
                            "(k p) l -> p k l", p=128)[:, :, lc * 512:(lc + 1) * 512])

                    wadaA_t = [pqw.tile([128, CT, 256], BF16, tag="wadaAp",
                                        bufs=4, name=f"wadaA{i}") for i in range(8)]

                    # ---- DMA issue order (SP FIFO) ----
                    dma(out=c_tmod, in_=tmodT[:, :])
                    dma(out=c_adab, in_=adabT[:, :])
                    dma(out=c_n1, in_=n1w[:, :])
                    dma(out=c_n2, in_=n2w[:, :])
                    dma(out=c_n3, in_=n3w[:, :])
                    x_fetch(0)
                    x_fetch(1)
                    for i in range(8):
                        dma(out=wadaA_t[i], in_=wadaA[:, :, i * 256:(i + 1) * 256])
                    dma(out=c_cs4, in_=cs4[:, :])
                    dma(out=c_sc4, in_=sc4[:, :])
                    dma(out=w_q, in_=wq2[:, :, :, :])
                    dma(out=w_qs, in_=wqs2[:, :, :, :])

                    mark("norm1")
                    # ---- silu(t_mod) ----
                    sg_t = pqw.tile([128, CT], FP32, tag="sg_t")
                    nc.scalar.activation(out=sg_t, in_=c_tmod,
                                         func=mybir.ActivationFunctionType.Sigmoid)
                    nc.vector.tensor_mul(silu_bf, sg_t, c_tmod)

                    pbs = {}

                    def norm1_ssq(lc):
                        pssq = ps.tile([1, 512], FP32, tag="pB", bufs=2,
                                       name=f"pssq{lc}")
                        for k in range(CT):
                            xsq = pqw.tile([128, 512], BF16, tag="xsq", bufs=2,
                                           name=f"xsq{lc}_{k}")
                            nc.vector.tensor_mul(xsq, xc[lc][:, k, :], xc[lc][:, k, :])
                            nc.tensor.matmul(pssq, ones_col, xsq,
                                             start=(k == 0), stop=(k == CT - 1))
                        rstd = pqw.tile([1, 512], FP32, tag="rstd", bufs=1,
                                        name=f"rstd{lc}")
                        nc.scalar.activation(out=rstd, in_=pssq,
                                             func=mybir.ActivationFunctionType.Sqrt,
                                             bias=eps_c, scale=1.0 / C)
                        nc.vector.reciprocal(rstd, rstd)
                        rstd_bf = pqw.tile([1, 512], BF16, tag="rstd_bf", bufs=1,
                                           name=f"rstdb{lc}")
                        nc.vector.tensor_copy(rstd_bf, rstd)
                        pb = ps.tile([128, 512], FP32, tag="pA", bufs=4,
                                     name=f"pbn1{lc}")
                        nc.tensor.matmul(pb, ones_row, rstd_bf, start=True, stop=True)
                        pbs[lc] = pb

                    def mod1(lc):
                        sl = slice(lc * 512, (lc + 1) * 512)
                        for k in range(CT):
                            dst = xsa2[k // 2][:, k % 2, sl]
                            nc.vector.scalar_tensor_tensor(
                                out=dst, in0=xc[lc][:, k, :],
                                scalar=w1eff[:, k:k + 1], in1=pbs[lc],
                                op0=mybir.AluOpType.mult,
                                op1=mybir.AluOpType.mult)
                            nc.scalar.activation(
                                out=dst, in_=dst,
                                func=mybir.ActivationFunctionType.Identity,
                                bias=sh_sa(k))

                    norm1_ssq(0)
                    norm1_ssq(1)

                    mark("adaLN")
                    # ---- adaLN part A: shift_sa + scale_sa (weight-stationary) ----
                    pmA = ps.tile([128, 16], FP32, tag="pB", bufs=2, name="pmA")
                    for j in range(16):
                        for k in range(CT):
                            nc.tensor.matmul(pmA[:, j:j + 1],
                                             wadaA_t[j // 2][:, k,
                                                             (j % 2) * 128:(j % 2 + 1) * 128],
                                             silu_bf[:, k:k + 1],
                                             start=(k == 0), stop=(k == CT - 1))
                    nc.vector.tensor_add(modsT[:, 0:16], pmA, c_adab[:, 0:16])
                    nc.vector.tensor_scalar(out=w1eff, in0=modsT[:, 8:16],
                                            scalar1=1.0, scalar2=None,
                                            op0=mybir.AluOpType.add)
                    nc.vector.tensor_mul(w1eff, w1eff, c_n1)

                    mark("mod1")
                    # ---- modulate -> xsa2 fp8 DR-paired ----
                    mod1(0)
                    x_fetch(2)
                    mod1(1)
                    x_fetch(3)
                    dma(out=xres, in_=xq_f[:, :].rearrange(
                        "(k p) l -> p k l", p=128))
                    norm1_ssq(2)
                    mod1(2)
                    norm1_ssq(3)
                    mod1(3)

                    def proj_dr(out_psum, w, m, xcols, nkp=KP):
                        for kp in range(nkp):
                            nc.tensor.matmul(out_psum,
                                             w[:, kp, :, m * 128:(m + 1) * 128],
                                             xcols(kp),
                                             start=(kp == 0), stop=(kp == nkp - 1),
                                             perf_mode=DR)

                    def rope_apply(dst, pk_, pks_, cols):
                        kb = pp.tile([128, 512], BF16, tag="ropet", bufs=6, name="kb")
                        nc.scalar.copy(out=kb, in_=pk_)
                        m1 = pp.tile([128, 512], BF16, tag="ropet", bufs=6, name="m1")
                        nc.vector.tensor_mul(m1, kb, c_cs4[:, cols])
                        m2 = pp.tile([128, 512], BF16, tag="ropet", bufs=6, name="m2")
                        nc.vector.tensor_mul(m2, pks_, c_sc4[:, cols])
                        nc.gpsimd.tensor_add(dst, m1, m2)

                    mark("q_proj")
                    # =========== q projection (own chunk) + rope ===========
                    OWN = slice(0, LQ)
                    for m in range(CT):
                        pq = ps.tile([128, LQ], FP32, tag="pA", bufs=4, name=f"pq{m}")
                        proj_dr(pq, w_q, m, lambda kp: xsa2[kp][:, :, OWN])
                        pqs = ps.tile([128, LQ], FP32, tag="pA", bufs=4, name=f"pqs{m}")
                        proj_dr(pqs, w_qs, m, lambda kp: xsa2[kp][:, :, OWN])
                        rope_apply(qT[m], pq, pqs, OWN)

                    mark("k_proj")
                    # =========== k projection (full L) + rope ===========
                    w_k = pqw.tile([128, KP, 2, C], F8, tag="wmain", bufs=1, name="w_k")
                    w_ks = pqw.tile([128, KP, 2, C], F8, tag="wswap", bufs=1, name="w_ks")
                    w_v = pqw.tile([128, KP, 2, C], F8, tag="wmain", bufs=1, name="w_v")
                    dma(out=w_k, in_=wk2[:, :, :, :])
                    dma(out=w_ks, in_=wks2[:, :, :, :])
                    dma(out=w_v, in_=wv2[:, :, :, :])
                    for m in range(CT):
                        for lc in range(LCH):
                            sl = slice(lc * 512, (lc + 1) * 512)
                            pk = ps.tile([128, 512], FP32, tag="pA", bufs=4,
                                         name=f"pk{m}_{lc}")
                            proj_dr(pk, w_k, m, lambda kp: xsa2[kp][:, :, sl])
                            pks = ps.tile([128, 512], FP32, tag="pA", bufs=4,
                                          name=f"pks{m}_{lc}")
                            proj_dr(pks, w_ks, m, lambda kp: xsa2[kp][:, :, sl])
                            rope_apply(kT[m][:, sl], pk, pks, sl)

                    mark("v_proj")
                    # =========== v projection (full L), natural + ones col ===========
                    for t in range(LKT):
                        nc.vector.memset(vsb[t][:, :, D:D + 1], 1.0)
                        for g in range(2):
                            pv = ps.tile([128, 512], FP32, tag="pA", bufs=4,
                                         name=f"pv{t}_{g}")
                            for kp in range(KP):
                                nc.tensor.matmul(
                                    pv, xsa2[kp][:, :, t * 128:(t + 1) * 128],
                                    w_v[:, kp, :, g * 512:(g + 1) * 512],
                                    start=(kp == 0), stop=(kp == KP - 1),
                                    perf_mode=DR)
                            nc.scalar.copy(
                                out=vsb[t][:, g * 8:(g + 1) * 8, 0:D],
                                in_=pv.rearrange("p (h d) -> p h d", h=8))

                # pqw closed: qkv weights + xsa2 freed
                # weights/data needed during + after self-attn
                pat_cm = tc.tile_pool(name="pat", bufs=1)
                pat = pat_cm.__enter__()
                w_sa = pat.tile([128, KP, 2, C], F8, tag="w_sa")
                w_kv = pat.tile([128, 3, 2, 2 * C], F8, tag="w_kv")
                a_t = pat.tile([128, 3, 2, L2], F8, tag="a_t")
                dma(out=w_sa, in_=wsa2[:, :, :, :])
                dma(out=w_kv, in_=wkv2[:, :, :, :])
                dma(out=a_t, in_=aud2[:, :, :, :])
                wadaB_t = {}

                def adaB_fetch(i):
                    wadaB_t[i] = pat.tile([128, CT, 512], BF16, tag="wadaB",
                                          bufs=4, name=f"wadaB{i}")
                    dma(out=wadaB_t[i], in_=wadaB[i])

                def cross_kv_piece(i):
                    # i in 0..11: 8 kc tiles then 4 vc tiles
                    if i < 8:
                        m = i
                        pkc = ps.tile([128, L2], FP32, tag="pB", bufs=2, name=f"pkc{m}")
                        for kp in range(3):
                            nc.tensor.matmul(pkc,
                                             w_kv[:, kp, :, m * 128:(m + 1) * 128],
                                             a_t[:, kp, :, :],
                                             start=(kp == 0), stop=(kp == 2),
                                             perf_mode=DR)
                        nc.vector.tensor_copy(kcT[m], pkc)
                    else:
                        t = i - 8
                        nc.vector.memset(vcb[t][:, :, D:D + 1], 1.0)
                        for g in range(2):
                            pvc = ps.tile([128, 512], FP32, tag="pB", bufs=2,
                                          name=f"pvc{t}_{g}")
                            for kp in range(3):
                                nc.tensor.matmul(
                                    pvc, a_t[:, kp, :, t * 128:(t + 1) * 128],
                                    w_kv[:, kp, :, C + g * 512:C + (g + 1) * 512],
                                    start=(kp == 0), stop=(kp == 2),
                                    perf_mode=DR)
                            nc.vector.tensor_copy(
                                vcb[t][:, g * 8:(g + 1) * 8, 0:D],
                                pvc.rearrange("p (h d) -> p h d", h=8))

                def adaB_piece(i):
                    # modsT cols 16+4i .. 20+4i
                    j0 = 16 + 4 * i
                    pmB = ps.tile([128, 4], FP32, tag="pB", bufs=2, name=f"pmB{i}")
                    for jj in range(4):
                        for k in range(CT):
                            nc.tensor.matmul(pmB[:, jj:jj + 1],
                                             wadaB_t[i][:, k, jj * 128:(jj + 1) * 128],
                                             silu_bf[:, k:k + 1],
                                             start=(k == 0), stop=(k == CT - 1))
                    nc.vector.tensor_add(modsT[:, j0:j0 + 4], pmB,
                                         c_adab[:, j0:j0 + 4])
                    if i == 5:
                        nc.vector.tensor_scalar(out=w3eff, in0=modsT[:, 32:40],
                                                scalar1=1.0, scalar2=None,
                                                op0=mybir.AluOpType.add)
                        nc.vector.tensor_mul(w3eff, w3eff, c_n3)

                mark("self_attn")
                # =========== self-attention ===========
                for h in range(H):
                    m = h // 2
                    rs = slice((h % 2) * 64, (h % 2) * 64 + 64)
                    po = ps.tile([65, LQ], FP32, tag="pC", bufs=2, name=f"po{h}")
                    pexps = []
                    LAG = 3
                    for t in range(LKT):
                        psc = ps.tile([128, LQ], FP32, tag="pA", bufs=4,
                                      name=f"psc{h}_{t}")
                        nc.tensor.matmul(psc, kT[m][rs, t * 128:(t + 1) * 128],
                                         qT[m][rs, :], start=True, stop=True)
                        pexp = pp.tile([128, LQ], BF16, tag="pexp", bufs=6,
                                       name=f"pexp{h}_{t}")
                        nc.scalar.activation(out=pexp, in_=psc,
                                             func=mybir.ActivationFunctionType.Exp,
                                             scale=KSC)
                        pexps.append(pexp)
                        if t >= LAG:
                            tt = t - LAG
                            nc.tensor.matmul(po, vsb[tt][:, h, :], pexps[tt],
                                             start=(tt == 0), stop=False)
                    for tt in range(LKT - LAG, LKT):
                        nc.tensor.matmul(po, vsb[tt][:, h, :], pexps[tt],
                                         start=False, stop=(tt == LKT - 1))
                    rec = pp.tile([1, LQ], FP32, tag="rec", bufs=2, name=f"rec{h}")
                    nc.vector.reciprocal(rec, po[64:65, :])
                    rec_bf = pp.tile([1, LQ], BF16, tag="rec_bf", bufs=2, name=f"recb{h}")
                    nc.vector.tensor_copy(rec_bf, rec)
                    pbc = ps.tile([64, LQ], FP32, tag="pA", bufs=4, name=f"pbc{h}")
                    nc.tensor.matmul(pbc, ones_row[:, 0:64], rec_bf,
                                     start=True, stop=True)
                    rb_sb = pp.tile([64, LQ], BF16, tag="rb_sb", bufs=2,
                                    name=f"rb{h}")
                    nc.vector.tensor_copy(rb_sb, pbc)
                    nc.vector.tensor_mul(att2[m // 2][rs, m % 2, :], po[0:64, :], rb_sb)
                    # fill PE idle in the Act(exp)-bound window
                    if h < 8:
                        adaB_fetch(h)
                    if 2 <= h < 14:
                        cross_kv_piece(h - 2)
                    if h >= 8:
                        adaB_piece(h - 8)

                mark("sa_out")
                # =========== self-attn out proj + gated residual ===========
                for m in range(CT):
                    pso = ps.tile([128, LQ], FP32, tag="pA", bufs=4, name=f"pso{m}")
                    proj_dr(pso, w_sa, m, lambda kp: att2[kp][:, :, :])
                    nc.vector.scalar_tensor_tensor(
                        out=xres[:, m, :], in0=pso, scalar=g_sa(m), in1=xres[:, m, :],
                        op0=mybir.AluOpType.mult, op1=mybir.AluOpType.add)
                pat_cm.__exit__(None, None, None)

            mark("cross")
            # =========== cross attention + MLP ===========
            with tc.tile_pool(name="pca", bufs=1) as pca:
                w_qc = pca.tile([128, KP, 2, C], F8, tag="w_qc")
                dma(out=w_qc, in_=wqc2[:, :, :, :])
                w_ca = pca.tile([128, KP, 2, C], F8, tag="w_ca")
                dma(out=w_ca, in_=wca2[:, :, :, :])
                # MLP gate/up weight stream (2 mg ahead)
                wgh_t, wgl_t, wuh_t, wul_t = {}, {}, {}, {}

                def gu_fetch(mg):
                    for d, src_, nm in ((wgh_t, wgh, "gh"), (wgl_t, wgl, "gl"),
                                        (wuh_t, wuh, "uh"), (wul_t, wul, "ul")):
                        d[mg] = pca.tile([128, KP, 2, 512], F8, tag="wgu", bufs=8,
                                         name=f"w{nm}{mg}")
                        dma(out=d[mg], in_=src_[mg])

                gu_fetch(0)
                gu_fetch(1)

                # norm2 (no modulation) -> xnb2 fp8 DR-paired
                pssq = ps.tile([1, LQ], FP32, tag="pB", bufs=2, name="pssq_n2")
                xnb2 = [pca.tile([128, 2, LQ], F8, tag=f"xn{j}", name=f"xnb{j}")
                        for j in range(KP)]
                for k in range(CT):
                    xsq = pca.tile([128, LQ], BF16, tag="xsq2", bufs=1, name=f"xsq2_{k}")
                    nc.vector.tensor_mul(xsq, xres[:, k, :], xres[:, k, :])
                    nc.tensor.matmul(pssq, ones_col, xsq,
                                     start=(k == 0), stop=(k == CT - 1))
                rstd = pca.tile([1, LQ], FP32, tag="rstd", bufs=1, name="rstd_n2")
                nc.scalar.activation(out=rstd, in_=pssq,
                                     func=mybir.ActivationFunctionType.Sqrt,
                                     bias=eps_c, scale=1.0 / C)
                nc.vector.reciprocal(rstd, rstd)
                rstd_bf = pca.tile([1, LQ], BF16, tag="rstd_bf", bufs=1, name="rstdb_n2")
                nc.vector.tensor_copy(rstd_bf, rstd)
                pb2 = ps.tile([128, LQ], FP32, tag="pA", bufs=4, name="pb_n2")
                nc.tensor.matmul(pb2, ones_row, rstd_bf, start=True, stop=True)
                for k in range(CT):
                    nc.vector.scalar_tensor_tensor(
                        out=xnb2[k // 2][:, k % 2, :], in0=xres[:, k, :],
                        scalar=c_n2[:, k:k + 1], in1=pb2,
                        op0=mybir.AluOpType.mult, op1=mybir.AluOpType.mult)

                def proj_dr2(out_psum, w, m, xcols, nkp=KP):
                    for kp in range(nkp):
                        nc.tensor.matmul(out_psum,
                                         w[:, kp, :, m * 128:(m + 1) * 128],
                                         xcols(kp),
                                         start=(kp == 0), stop=(kp == nkp - 1),
                                         perf_mode=DR)

                # cross q projection
                qcT = [pca.tile([128, LQ], BF16, tag=f"qc{m}", name=f"qcT{m}")
                       for m in range(CT)]
                for m in range(CT):
                    pq = ps.tile([128, LQ], FP32, tag="pA", bufs=4, name=f"pqc{m}")
                    proj_dr2(pq, w_qc, m, lambda kp: xnb2[kp][:, :, :])
                    nc.vector.tensor_copy(qcT[m], pq)

                mark("cross_attn")
                # attention over audio; stream remaining MLP weights meanwhile
                for h in range(H):
                    m = h // 2
                    rs = slice((h % 2) * 64, (h % 2) * 64 + 64)
                    po = ps.tile([65, LQ], FP32, tag="pC", bufs=2, name=f"poc{h}")
                    pexps = []
                    for t in range(4):
                        psc = ps.tile([128, LQ], FP32, tag="pA", bufs=4,
                                      name=f"pscc{h}_{t}")
                        nc.tensor.matmul(psc, kcT[m][rs, t * 128:(t + 1) * 128],
                                         qcT[m][rs, :], start=True, stop=True)
                        pexp = pp.tile([128, LQ], BF16, tag="pexp", bufs=6,
                                       name=f"pexpc{h}_{t}")
                        nc.scalar.activation(out=pexp, in_=psc,
                                             func=mybir.ActivationFunctionType.Exp,
                                             scale=KSC)
                        pexps.append(pexp)
                        if t >= 2:
                            nc.tensor.matmul(po, vcb[t - 2][:, h, :], pexps[t - 2],
                                             start=(t == 2), stop=False)
                    for tt in (2, 3):
                        nc.tensor.matmul(po, vcb[tt][:, h, :], pexps[tt],
                                         start=False, stop=(tt == 3))
                    rec = pp.tile([1, LQ], FP32, tag="rec", bufs=2, name=f"recc{h}")
                    nc.vector.reciprocal(rec, po[64:65, :])
                    rec_bf = pp.tile([1, LQ], BF16, tag="rec_bf", bufs=2,
                                     name=f"recbc{h}")
                    nc.vector.tensor_copy(rec_bf, rec)
                    pbc = ps.tile([64, LQ], FP32, tag="pA", bufs=4, name=f"pbcc{h}")
                    nc.tensor.matmul(pbc, ones_row[:, 0:64], rec_bf,
                                     start=True, stop=True)
                    rb_sb = pp.tile([64, LQ], BF16, tag="rb_sb", bufs=2,
                                    name=f"rbc{h}")
                    nc.vector.tensor_copy(rb_sb, pbc)
                    nc.vector.tensor_mul(att2[m // 2][rs, m % 2, :], po[0:64, :], rb_sb)
                    if h % 3 == 0 and 2 + h // 3 < 8:
                        gu_fetch(2 + h // 3)

                mark("ca_out")
                # cross out proj + residual (no gate)
                for m in range(CT):
                    pco = ps.tile([128, LQ], FP32, tag="pA", bufs=4, name=f"pcao{m}")
                    proj_dr2(pco, w_ca, m, lambda kp: att2[kp][:, :, :])
                    nc.vector.tensor_add(xres[:, m, :], xres[:, m, :], pco)

                mark("mlp_norm")
                # norm3 + modulation -> bf16, then hi/lo fp8 split
                pssq3 = ps.tile([1, LQ], FP32, tag="pB", bufs=2, name="pssq_n3")
                xmb = [pca.tile([128, LQ], BF16, tag=f"xm{k}", name=f"xmb{k}")
                       for k in range(CT)]
                xh2 = [pca.tile([128, 2, LQ], F8, tag=f"xh{j}", name=f"xh{j}")
                       for j in range(KP)]
                xl2 = [pca.tile([128, 2, LQ], F8, tag=f"xl{j}", name=f"xl{j}")
                       for j in range(KP)]
                x64 = [pca.tile([128, 2, LQ], F8, tag=f"x6{j}", name=f"x6{j}")
                       for j in range(KP)]
                for k in range(CT):
                    xsq = pca.tile([128, LQ], BF16, tag="xsq2", bufs=1, name=f"xsq3_{k}")
                    nc.vector.tensor_mul(xsq, xres[:, k, :], xres[:, k, :])
                    nc.tensor.matmul(pssq3, ones_col, xsq,
                                     start=(k == 0), stop=(k == CT - 1))
                rstd3 = pca.tile([1, LQ], FP32, tag="rstd", bufs=1, name="rstd_n3")
                nc.scalar.activation(out=rstd3, in_=pssq3,
                                     func=mybir.ActivationFunctionType.Sqrt,
                                     bias=eps_c, scale=1.0 / C)
                nc.vector.reciprocal(rstd3, rstd3)
                rstd3_bf = pca.tile([1, LQ], BF16, tag="rstd_bf", bufs=1,
                                    name="rstdb_n3")
                nc.vector.tensor_copy(rstd3_bf, rstd3)
                pb3 = ps.tile([128, LQ], FP32, tag="pA", bufs=4, name="pb_n3")
                nc.tensor.matmul(pb3, ones_row, rstd3_bf, start=True, stop=True)
                for k in range(CT):
                    nc.vector.scalar_tensor_tensor(
                        out=xmb[k], in0=xres[:, k, :], scalar=w3eff[:, k:k + 1],
                        in1=pb3,
                        op0=mybir.AluOpType.mult, op1=mybir.AluOpType.mult)
                    nc.scalar.activation(out=xmb[k], in_=xmb[k],
                                         func=mybir.ActivationFunctionType.Identity,
                                         bias=sh_ml(k))
                    hi = xh2[k // 2][:, k % 2, :]
                    lo = xl2[k // 2][:, k % 2, :]
                    nc.scalar.copy(out=hi, in_=xmb[k])
                    nc.vector.tensor_sub(lo, xmb[k], hi)
                    nc.scalar.activation(out=x64[k // 2][:, k % 2, :], in_=xmb[k],
                                         func=mybir.ActivationFunctionType.Identity,
                                         scale=1.0 / LOSC)

                mark("gate_up")
                # h2: fp8 DR-paired ffn activations
                h2 = [pca.tile([128, 2, LQ], F8, tag=f"h{t}", name=f"h2_{t}")
                      for t in range(FF // 256)]
                h64_2 = [pca.tile([128, 2, LQ], F8, tag=f"h6{t}", name=f"h64_{t}")
                         for t in range(FF // 256)]
                wdh_t, wdl_t = {}, {}

                def down_fetch(m):
                    wdh_t[m] = pca.tile([128, 16, 2, 128], F8, tag="wdw", bufs=4,
                                        name=f"wdh{m}")
                    dma(out=wdh_t[m], in_=wdh[m])
                    wdl_t[m] = pca.tile([128, 16, 2, 128], F8, tag="wdw", bufs=4,
                                        name=f"wdl{m}")
                    dma(out=wdl_t[m], in_=wdl[m])

                def dr_hilo(p1, wh, wl, mi):
                    # Xh*Wh + (X/64)*(Wl*64) + Xl*Wh, all at true scale
                    ms = slice(mi * 128, (mi + 1) * 128)
                    for kp in range(KP):
                        nc.tensor.matmul(p1, wh[:, kp, :, ms], xh2[kp][:, :, :],
                                         start=(kp == 0), stop=False, perf_mode=DR)
                    for kp in range(KP):
                        nc.tensor.matmul(p1, wl[:, kp, :, ms], x64[kp][:, :, :],
                                         start=False, stop=False, perf_mode=DR)
                    for kp in range(KP):
                        nc.tensor.matmul(p1, wh[:, kp, :, ms], xl2[kp][:, :, :],
                                         start=False, stop=(kp == KP - 1),
                                         perf_mode=DR)

                for mg in range(8):
                    if mg >= 6:
                        down_fetch(mg - 6)
                    for mi in range(4):
                        p1g = ps.tile([128, LQ], FP32, tag="pA", bufs=4,
                                      name=f"p1g{mg}_{mi}")
                        dr_hilo(p1g, wgh_t[mg], wgl_t[mg], mi)
                        sg = pca.tile([128, LQ], BF16, tag="sgb", bufs=2,
                                      name=f"sg{mg}_{mi}")
                        nc.scalar.activation(out=sg, in_=p1g,
                                             func=mybir.ActivationFunctionType.Sigmoid)
                        gbf = pca.tile([128, LQ], BF16, tag="gbf", bufs=4,
                                       name=f"gbf{mg}_{mi}")
                        nc.vector.tensor_mul(gbf, sg, p1g)
                        p1u = ps.tile([128, LQ], FP32, tag="pA", bufs=4,
                                      name=f"p1u{mg}_{mi}")
                        dr_hilo(p1u, wuh_t[mg], wul_t[mg], mi)
                        t = mg * 4 + mi
                        nc.vector.tensor_mul(h2[t // 2][:, t % 2, :], gbf, p1u)
                        h64 = h64_2[t // 2][:, t % 2, :]
                        nc.scalar.activation(
                            out=h64, in_=h2[t // 2][:, t % 2, :],
                            func=mybir.ActivationFunctionType.Identity,
                            scale=1.0 / LOSC)

                mark("down")
                # down proj: P1 = H*Wdh, P2 = H*Wdl(x64); out = (P1 + P2/64)*g + xres
                for m in range(CT):
                    if m + 2 < CT:
                        down_fetch(m + 2)
                    pd1 = ps.tile([128, LQ], FP32, tag="pA", bufs=4, name=f"pd1{m}")
                    for fp in range(16):
                        nc.tensor.matmul(pd1, wdh_t[m][:, fp, :, :],
                                         h2[fp][:, :, :],
                                         start=(fp == 0), stop=False,
                                         perf_mode=DR)
                    for fp in range(16):
                        nc.tensor.matmul(pd1, wdl_t[m][:, fp, :, :],
                                         h64_2[fp][:, :, :],
                                         start=False, stop=(fp == 15),
                                         perf_mode=DR)
                    of = pca.tile([128, LQ], FP32, tag="of", bufs=2, name=f"of{m}")
                    nc.vector.scalar_tensor_tensor(
                        out=of, in0=pd1, scalar=g_ml(m), in1=xres[:, m, :],
                        op0=mybir.AluOpType.mult, op1=mybir.AluOpType.add)
                    dma(out=outT[m * 128:(m + 1) * 128, :], in_=of)

    nc.compile()
    return nc


_ROPE_PERM = None
_SWAP_PERM = None


def _perms():
    global _ROPE_PERM, _SWAP_PERM
    if _ROPE_PERM is None:
        p = np.zeros(C, dtype=np.int64)
        s = np.zeros(C, dtype=np.int64)
        for h in range(H):
            for i in range(D // 2):
                p[h * D + i] = h * D + 2 * i               # real block
                p[h * D + D // 2 + i] = h * D + 2 * i + 1  # imag block
                s[h * D + i] = h * D + 2 * i + 1           # swapped: imag first
                s[h * D + D // 2 + i] = h * D + 2 * i
        _ROPE_PERM, _SWAP_PERM = p, s
    return _ROPE_PERM, _SWAP_PERM


def _bf(a):
    return np.ascontiguousarray(a).astype(ml_dtypes.bfloat16)


def _f8(a):
    return np.ascontiguousarray(a).astype(F8NP)


def _dr_pack(W):
    # [n_in, n_out] -> [128, n_in//256, 2, n_out]
    n_in, n_out = W.shape
    kp = n_in // 256
    return W.reshape(kp, 2, 128, n_out).transpose(2, 0, 1, 3)


def _hilo(W):
    hi = W.astype(F8NP)
    lo = ((W - hi.astype(np.float32)) * LOSC).astype(F8NP)
    return hi, lo


def _prep_shared(W_qkv, W_sa_out, W_q, W_kv, W_ca_out, W_gate, W_up, W_down,
                 adaLN_W, adaLN_b, norm1_w, norm2_w, norm3_w):
    perm, sperm = _perms()
    wq = W_qkv[:, 0:C][:, perm]
    wqs = W_qkv[:, 0:C][:, sperm]
    wk = W_qkv[:, C:2 * C][:, perm]
    wks = W_qkv[:, C:2 * C][:, sperm]
    wv = W_qkv[:, 2 * C:3 * C]

    def pack8(W):
        return _f8(_dr_pack(np.asarray(W, np.float32)))

    wgh_, wgl_ = _hilo(np.asarray(W_gate, np.float32))
    wuh_, wul_ = _hilo(np.asarray(W_up, np.float32))
    wdh_, wdl_ = _hilo(np.asarray(W_down, np.float32))

    def mlp_pack(w8):  # fp8 [C, FF] -> [8 mg][128, kp, 2, 512]
        d = _dr_pack(w8.astype(np.float32)).astype(F8NP)  # [128, 4, 2, 4096]
        return np.ascontiguousarray(d.reshape(128, KP, 2, 8, 512)
                                    .transpose(3, 0, 1, 2, 4))

    def down_pack(w8):  # fp8 [FF, C] -> [8 m][128, 16 fp, 2, 128]
        d = _dr_pack(w8.astype(np.float32)).astype(F8NP)  # [128, 16, 2, C]
        return np.ascontiguousarray(d.reshape(128, 16, 2, CT, 128)
                                    .transpose(3, 0, 1, 2, 4))

    # adaLN weight-stationary tiles: [p, k, j*128+q] = W[128k+p, 128j+q]
    wada = np.asarray(adaLN_W, np.float32).reshape(CT, 128, 48, 128)
    wadaA_h = wada[:, :, 0:16, :].transpose(1, 0, 2, 3).reshape(128, CT, 2048)
    wadaB_h = np.stack([
        wada[:, :, 16 + 4 * i:20 + 4 * i, :].transpose(1, 0, 2, 3)
        .reshape(128, CT, 512) for i in range(8)])

    sh = {
        "wq2": pack8(wq), "wqs2": pack8(wqs), "wk2": pack8(wk),
        "wks2": pack8(wks), "wv2": pack8(wv),
        "wsa2": pack8(W_sa_out), "wqc2": pack8(W_q), "wkv2": pack8(W_kv),
        "wca2": pack8(W_ca_out),
        "wgh": mlp_pack(wgh_), "wgl": mlp_pack(wgl_),
        "wuh": mlp_pack(wuh_), "wul": mlp_pack(wul_),
        "wdh": down_pack(wdh_), "wdl": down_pack(wdl_),
        "wadaA": _bf(wadaA_h), "wadaB": _bf(wadaB_h),
        "adabT": np.ascontiguousarray(
            np.asarray(adaLN_b, np.float32).reshape(48, 128).T),
        "n1w": np.ascontiguousarray(
            np.asarray(norm1_w, np.float32).reshape(8, 128).T),
        "n2w": np.ascontiguousarray(
            np.asarray(norm2_w, np.float32).reshape(8, 128).T),
        "n3w": np.ascontiguousarray(
            np.asarray(norm3_w, np.float32).reshape(8, 128).T),
    }
    return sh


def make_in_maps(x, t_mod, audio_context, freqs_cos, freqs_sin,
                 norm1_w, norm2_w, norm3_w,
                 W_qkv, W_sa_out, W_q, W_kv, W_ca_out,
                 W_gate, W_up, W_down, adaLN_W, adaLN_b):
    sh = _prep_shared(W_qkv, W_sa_out, W_q, W_kv, W_ca_out, W_gate, W_up,
                      W_down, adaLN_W, adaLN_b, norm1_w, norm2_w, norm3_w)
    cosT = np.ascontiguousarray(np.asarray(freqs_cos, np.float32).T)
    sinT = np.ascontiguousarray(np.asarray(freqs_sin, np.float32).T)

    in_maps = []
    for core in range(NCORE):
        b, j = divmod(core, 4)
        # roll the token axis so this core's own 512 tokens sit at [0, LQ)
        xT = np.roll(np.ascontiguousarray(np.asarray(x, np.float32)[b].T),
                     -j * LQ, axis=1)
        m = dict(sh)
        m["x_bf"] = _bf(xT)
        m["xq_f"] = np.ascontiguousarray(xT[:, 0:LQ])
        cr = np.roll(cosT, -j * LQ, axis=1)
        sr = np.roll(sinT, -j * LQ, axis=1)
        m["cs4"] = _bf(np.concatenate([cr, cr, cr, cr], axis=0))
        m["sc4"] = _bf(np.concatenate([-sr, sr, -sr, sr], axis=0))
        m["aud2"] = _f8(_dr_pack(
            np.ascontiguousarray(np.asarray(audio_context, np.float32)[b].T)))
        m["tmodT"] = np.ascontiguousarray(
            np.asarray(t_mod, np.float32)[b].reshape(8, 128).T)
        in_maps.append(m)
    return in_maps


_NC_CACHE = None


def _get_nc():
    global _NC_CACHE
    if _NC_CACHE is None:
        _NC_CACHE = build_bass()
    return _NC_CACHE


def kernel(**inputs):
    nc = _get_nc()
    inputs = {k: np.asarray(v) for k, v in inputs.items()}
    in_maps = make_in_maps(**inputs)
    res = run_bass_kernel_spmd(nc, in_maps, list(range(NCORE)))
    out = np.zeros((B, L, C), np.float32)
    for core in range(NCORE):
        b, j = divmod(core, 4)
        out[b, j * LQ:(j + 1) * LQ, :] = res.results[core]["outT"].T
    return out


# revision 21
# speedup vs baseline: 1.5752x; 1.1181x over previous
"""Trainium2 Bass kernel for nn_ExpressionModel (dense DiT-style transformer block).

Sharding: 8 cores = 2 (batch) x 4 (sequence chunks of 512 tokens).
Each core computes the full block for its 512 query tokens; K/V projections
for the full 2048-token batch are duplicated across the 4 cores of a batch
(no collectives needed).

Residual stream is transposed (channels on partitions). All dense
projections run in fp8e4 with DoubleRow perf mode (two contraction rows per
PE pass); the MLP uses hi+lo fp8 splitting (T ~ T_hi + T_lo/64) for both
weights and activations on gate/up, and for weights on down, to stay inside
the error budget. Attention scores / probabilities / p@V stay bf16.
RoPE is computed from two projections (natural + host-swapped weights) so
no engine shuffles partitions: k_rope = pk*cos + pks*sin_signed — two DVE
muls (PSUM direct) + one Pool add. adaLN runs weight-stationary (1-column
matmuls, ~free on PE); only shift/scale_sa loads up front, the other 32
columns stream in during self-attention.
"""

import numpy as np
import ml_dtypes

import concourse.bass as bass
import concourse.tile as tile
from concourse import bacc, mybir
from concourse.bass_utils import run_bass_kernel_spmd

FP32 = mybir.dt.float32
BF16 = mybir.dt.bfloat16
F8 = mybir.dt.float8e4
DR = mybir.MatmulPerfMode.DoubleRow
F8NP = ml_dtypes.float8_e4m3

STAGE_MARKS = []  # (instruction-id watermark, stage name) — profiling aid

B, L, C = 2, 2048, 1024
H, D = 16, 64
L2, TD = 512, 768
FF = 4096
EPS = 1e-6
NCORE = 8
LQ = 512            # query tokens per core
CT = C // 128       # 8 C partition-tiles
KP = C // 256       # 4 DoubleRow contraction pairs over C
LKT = L // 128      # 16 key chunks (self)
LCH = L // 512      # 4 512-token chunks
KSC = 1.0 / 8.0     # 1/sqrt(D)
LOSC = 64.0         # hi/lo split scale


def build_bass():
    nc = bacc.Bacc("TRN2", target_bir_lowering=False, debug=False)
    STAGE_MARKS.clear()

    def mark(stage):
        STAGE_MARKS.append((nc.next_id(), stage))

    def dma(out, in_):
        return nc.sync.dma_start(out=out, in_=in_)

    def din(name, shape, dt):
        return nc.dram_tensor(name, list(shape), dt, kind="ExternalInput")

    # --- inputs ---
    x_bf = din("x_bf", (C, L), BF16)            # x[b].T, bf16
    xq_f = din("xq_f", (C, LQ), FP32)           # own-chunk x[b].T, fp32 residual
    aud2 = din("aud2", (128, 3, 2, L2), F8)     # audio.T fp8 DR-paired
    tmodT = din("tmodT", (128, CT), FP32)
    cs4 = din("cs4", (128, L), BF16)            # [c;c;c;c] rows
    sc4 = din("sc4", (128, L), BF16)            # [-s;+s;-s;+s] rows
    adabT = din("adabT", (128, 48), FP32)
    n1w = din("n1w", (128, CT), FP32)
    n2w = din("n2w", (128, CT), FP32)
    n3w = din("n3w", (128, CT), FP32)
    wadaA = din("wadaA", (128, CT, 2048), BF16)   # adaLN W cols j0..15
    wadaB = din("wadaB", (8, 128, CT, 512), BF16)  # adaLN W cols j16..47, 8 pieces
    wq2 = din("wq2", (128, KP, 2, C), F8)       # W_qkv q block, rope-permuted, DR-paired
    wqs2 = din("wqs2", (128, KP, 2, C), F8)     # q block, swap-permuted
    wk2 = din("wk2", (128, KP, 2, C), F8)
    wks2 = din("wks2", (128, KP, 2, C), F8)
    wv2 = din("wv2", (128, KP, 2, C), F8)
    wsa2 = din("wsa2", (128, KP, 2, C), F8)
    wqc2 = din("wqc2", (128, KP, 2, C), F8)
    wkv2 = din("wkv2", (128, 3, 2, 2 * C), F8)
    wca2 = din("wca2", (128, KP, 2, C), F8)
    wgh = din("wgh", (8, 128, KP, 2, 512), F8)  # MLP weights hi/lo fp8
    wgl = din("wgl", (8, 128, KP, 2, 512), F8)
    wuh = din("wuh", (8, 128, KP, 2, 512), F8)
    wul = din("wul", (8, 128, KP, 2, 512), F8)
    wdh = din("wdh", (CT, 128, 16, 2, 128), F8)   # W_down hi, per out C-tile
    wdl = din("wdl", (CT, 128, 16, 2, 128), F8)

    outT = nc.dram_tensor("outT", [C, LQ], FP32, kind="ExternalOutput")

    with tile.TileContext(nc) as tc:
        with (
            tc.tile_pool(name="pp", bufs=1) as pp,              # persistent
            tc.tile_pool(name="ps", bufs=1, space="PSUM") as ps,
        ):
            # ---- persistent constants ----
            c_tmod = pp.tile([128, CT], FP32, tag="c_tmod")
            c_adab = pp.tile([128, 48], FP32, tag="c_adab")
            c_n1 = pp.tile([128, CT], FP32, tag="c_n1")
            c_n2 = pp.tile([128, CT], FP32, tag="c_n2")
            c_n3 = pp.tile([128, CT], FP32, tag="c_n3")
            c_cs4 = pp.tile([128, L], BF16, tag="c_cs4")
            c_sc4 = pp.tile([128, L], BF16, tag="c_sc4")
            xres = pp.tile([128, CT, LQ], FP32, tag="xres")
            ones_col = pp.tile([128, 1], BF16, tag="ones_col")
            ones_row = pp.tile([1, 128], BF16, tag="ones_row")
            eps_c = pp.tile([1, 1], FP32, tag="eps_c")
            nc.gpsimd.memset(ones_col, 1.0)
            nc.gpsimd.memset(ones_row, 1.0)
            nc.gpsimd.memset(eps_c, EPS)
            modsT = pp.tile([128, 48], FP32, tag="modsT")
            silu_bf = pp.tile([128, CT], BF16, tag="silu_bf")
            w1eff = pp.tile([128, CT], FP32, tag="w1eff")
            w3eff = pp.tile([128, CT], FP32, tag="w3eff")
            # attn output accumulators (fp8, DR-paired; reused by cross attn)
            att2 = [pp.tile([128, 2, LQ], F8, tag=f"att{j}", name=f"att{j}")
                    for j in range(KP)]
            # cross K (transposed) / V (natural), filled during self-attn
            kcT = [pp.tile([128, L2], BF16, tag=f"kc{m}", name=f"kcT{m}")
                   for m in range(CT)]
            vcb = [pp.tile([128, H, D + 1], BF16, tag=f"vc{t}", name=f"vcb{t}")
                   for t in range(4)]

            def sh_sa(k):
                return modsT[:, 0 + k:1 + k]

            def g_sa(k):
                return modsT[:, 16 + k:17 + k]

            def sh_ml(k):
                return modsT[:, 24 + k:25 + k]

            def g_ml(k):
                return modsT[:, 40 + k:41 + k]

            with tc.tile_pool(name="pkv", bufs=1) as pkv:
                kT = [pkv.tile([128, L], BF16, tag=f"kT{m}", name=f"kT{m}")
                      for m in range(CT)]
                vsb = [pkv.tile([128, H, D + 1], BF16, tag=f"v{t}", name=f"v{t}")
                       for t in range(LKT)]
                qT = [pkv.tile([128, LQ], BF16, tag=f"qT{m}", name=f"qT{m}")
                      for m in range(CT)]

                with tc.tile_pool(name="pqw", bufs=1) as pqw:
                    # qkv weights: q/k rotate one buffer, swaps likewise
                    w_q = pqw.tile([128, KP, 2, C], F8, tag="wmain", bufs=1, name="w_q")
                    w_qs = pqw.tile([128, KP, 2, C], F8, tag="wswap", bufs=1, name="w_qs")
                    xsa2 = [pqw.tile([128, 2, L], F8, tag=f"xsa{j}", name=f"xsa{j}")
                            for j in range(KP)]
                    # streamed x (4 chunks, 2 resident) and adaLN-A (2 pieces)
                    xc = {}

                    def x_fetch(lc):
                        xc[lc] = pqw.tile([128, CT, 512], BF16, tag="xinc",
                                          bufs=3, name=f"xin{lc}")
                        dma(out=xc[lc], in_=x_bf[:, :].rearrange(
                            "(k p) l -> p k l", p=128)[:, :, lc * 512:(lc + 1) * 512])

                    wadaA_t = [pqw.tile([128, CT, 256], BF16, tag="wadaAp",
                                        bufs=2, name=f"wadaA{i}") for i in range(8)]

                    # ---- DMA issue order (SP FIFO) ----
                    dma(out=c_tmod, in_=tmodT[:, :])
                    dma(out=c_adab, in_=adabT[:, :])
                    dma(out=c_n1, in_=n1w[:, :])
                    dma(out=c_n2, in_=n2w[:, :])
                    dma(out=c_n3, in_=n3w[:, :])
                    x_fetch(0)
                    x_fetch(1)
                    for i in range(8):
                        dma(out=wadaA_t[i], in_=wadaA[:, :, i * 256:(i + 1) * 256])
                    dma(out=c_cs4, in_=cs4[:, :])
                    dma(out=c_sc4, in_=sc4[:, :])
                    dma(out=w_q, in_=wq2[:, :, :, :])
                    dma(out=w_qs, in_=wqs2[:, :, :, :])

                    mark("norm1")
                    # ---- silu(t_mod) ----
                    sg_t = pqw.tile([128, CT], FP32, tag="sg_t")
                    nc.scalar.activation(out=sg_t, in_=c_tmod,
                                         func=mybir.ActivationFunctionType.Sigmoid)
                    nc.vector.tensor_mul(silu_bf, sg_t, c_tmod)

                    pbs = {}

                    def norm1_ssq(lc):
                        pssq = ps.tile([1, 512], FP32, tag="pB", bufs=2,
                                       name=f"pssq{lc}")
                        for k in range(CT):
                            xsq = pqw.tile([128, 512], BF16, tag="xsq", bufs=2,
                                           name=f"xsq{lc}_{k}")
                            nc.vector.tensor_mul(xsq, xc[lc][:, k, :], xc[lc][:, k, :])
                            nc.tensor.matmul(pssq, ones_col, xsq,
                                             start=(k == 0), stop=(k == CT - 1))
                        rstd = pqw.tile([1, 512], FP32, tag="rstd", bufs=1,
                                        name=f"rstd{lc}")
                        nc.scalar.activation(out=rstd, in_=pssq,
                                             func=mybir.ActivationFunctionType.Sqrt,
                                             bias=eps_c, scale=1.0 / C)
                        nc.vector.reciprocal(rstd, rstd)
                        rstd_bf = pqw.tile([1, 512], BF16, tag="rstd_bf", bufs=1,
                                           name=f"rstdb{lc}")
                        nc.vector.tensor_copy(rstd_bf, rstd)
                        pb = ps.tile([128, 512], FP32, tag="pA", bufs=4,
                                     name=f"pbn1{lc}")
                        nc.tensor.matmul(pb, ones_row, rstd_bf, start=True, stop=True)
                        pbs[lc] = pb

                    def mod1(lc):
                        sl = slice(lc * 512, (lc + 1) * 512)
                        for k in range(CT):
                            dst = xsa2[k // 2][:, k % 2, sl]
                            nc.vector.scalar_tensor_tensor(
                                out=dst, in0=xc[lc][:, k, :],
                                scalar=w1eff[:, k:k + 1], in1=pbs[lc],
                                op0=mybir.AluOpType.mult,
                                op1=mybir.AluOpType.mult)
                            nc.scalar.activation(
                                out=dst, in_=dst,
                                func=mybir.ActivationFunctionType.Identity,
                                bias=sh_sa(k))

                    norm1_ssq(0)
                    norm1_ssq(1)

                    mark("adaLN")
                    # ---- adaLN part A: shift_sa + scale_sa (weight-stationary) ----
                    pmA = ps.tile([128, 16], FP32, tag="pB", bufs=2, name="pmA")
                    for j in range(16):
                        for k in range(CT):
                            nc.tensor.matmul(pmA[:, j:j + 1],
                                             wadaA_t[j // 2][:, k,
                                                             (j % 2) * 128:(j % 2 + 1) * 128],
                                             silu_bf[:, k:k + 1],
                                             start=(k == 0), stop=(k == CT - 1))
                    nc.vector.tensor_add(modsT[:, 0:16], pmA, c_adab[:, 0:16])
                    nc.vector.tensor_scalar(out=w1eff, in0=modsT[:, 8:16],
                                            scalar1=1.0, scalar2=None,
                                            op0=mybir.AluOpType.add)
                    nc.vector.tensor_mul(w1eff, w1eff, c_n1)

                    mark("mod1")
                    # ---- modulate -> xsa2 fp8 DR-paired ----
                    mod1(0)
                    x_fetch(2)
                    mod1(1)
                    x_fetch(3)
                    dma(out=xres, in_=xq_f[:, :].rearrange(
                        "(k p) l -> p k l", p=128))
                    norm1_ssq(2)
                    mod1(2)
                    norm1_ssq(3)
                    mod1(3)

                    def proj_dr(out_psum, w, m, xcols, nkp=KP):
                        for kp in range(nkp):
                            nc.tensor.matmul(out_psum,
                                             w[:, kp, :, m * 128:(m + 1) * 128],
                                             xcols(kp),
                                             start=(kp == 0), stop=(kp == nkp - 1),
                                             perf_mode=DR)

                    def rope_apply(dst, pk_, pks_, cols):
                        kb = pp.tile([128, 512], BF16, tag="ropet", bufs=6, name="kb")
                        nc.scalar.copy(out=kb, in_=pk_)
                        m1 = pp.tile([128, 512], BF16, tag="ropet", bufs=6, name="m1")
                        nc.vector.tensor_mul(m1, kb, c_cs4[:, cols])
                        m2 = pp.tile([128, 512], BF16, tag="ropet", bufs=6, name="m2")
                        nc.vector.tensor_mul(m2, pks_, c_sc4[:, cols])
                        nc.gpsimd.tensor_add(dst, m1, m2)

                    mark("q_proj")
                    # =========== q projection (own chunk) + rope ===========
                    OWN = slice(0, LQ)
                    for m in range(CT):
                        pq = ps.tile([128, LQ], FP32, tag="pA", bufs=4, name=f"pq{m}")
                        proj_dr(pq, w_q, m, lambda kp: xsa2[kp][:, :, OWN])
                        pqs = ps.tile([128, LQ], FP32, tag="pA", bufs=4, name=f"pqs{m}")
                        proj_dr(pqs, w_qs, m, lambda kp: xsa2[kp][:, :, OWN])
                        rope_apply(qT[m], pq, pqs, OWN)

                    mark("k_proj")
                    # =========== k projection (full L) + rope ===========
                    w_k = pqw.tile([128, KP, 2, C], F8, tag="wmain", bufs=1, name="w_k")
                    w_ks = pqw.tile([128, KP, 2, C], F8, tag="wswap", bufs=1, name="w_ks")
                    w_v = pqw.tile([128, KP, 2, C], F8, tag="wmain", bufs=1, name="w_v")
                    dma(out=w_k, in_=wk2[:, :, :, :])
                    dma(out=w_ks, in_=wks2[:, :, :, :])
                    dma(out=w_v, in_=wv2[:, :, :, :])
                    for m in range(CT):
                        for lc in range(LCH):
                            sl = slice(lc * 512, (lc + 1) * 512)
                            pk = ps.tile([128, 512], FP32, tag="pA", bufs=4,
                                         name=f"pk{m}_{lc}")
                            proj_dr(pk, w_k, m, lambda kp: xsa2[kp][:, :, sl])
                            pks = ps.tile([128, 512], FP32, tag="pA", bufs=4,
                                          name=f"pks{m}_{lc}")
                            proj_dr(pks, w_ks, m, lambda kp: xsa2[kp][:, :, sl])
                            rope_apply(kT[m][:, sl], pk, pks, sl)

                    mark("v_proj")
                    # =========== v projection (full L), natural + ones col ===========
                    for t in range(LKT):
                        nc.vector.memset(vsb[t][:, :, D:D + 1], 1.0)
                        for g in range(2):
                            pv = ps.tile([128, 512], FP32, tag="pA", bufs=4,
                                         name=f"pv{t}_{g}")
                            for kp in range(KP):
                                nc.tensor.matmul(
                                    pv, xsa2[kp][:, :, t * 128:(t + 1) * 128],
                                    w_v[:, kp, :, g * 512:(g + 1) * 512],
                                    start=(kp == 0), stop=(kp == KP - 1),
                                    perf_mode=DR)
                            nc.scalar.copy(
                                out=vsb[t][:, g * 8:(g + 1) * 8, 0:D],
                                in_=pv.rearrange("p (h d) -> p h d", h=8))

                # pqw closed: qkv weights + xsa2 freed
                # weights/data needed during + after self-attn
                pat_cm = tc.tile_pool(name="pat", bufs=1)
                pat = pat_cm.__enter__()
                w_sa = pat.tile([128, KP, 2, C], F8, tag="w_sa")
                w_kv = pat.tile([128, 3, 2, 2 * C], F8, tag="w_kv")
                a_t = pat.tile([128, 3, 2, L2], F8, tag="a_t")
                dma(out=w_sa, in_=wsa2[:, :, :, :])
                dma(out=w_kv, in_=wkv2[:, :, :, :])
                dma(out=a_t, in_=aud2[:, :, :, :])
                wadaB_t = {}

                def adaB_fetch(i):
                    wadaB_t[i] = pat.tile([128, CT, 512], BF16, tag="wadaB",
                                          bufs=4, name=f"wadaB{i}")
                    dma(out=wadaB_t[i], in_=wadaB[i])

                def cross_kv_piece(i):
                    # i in 0..11: 8 kc tiles then 4 vc tiles
                    if i < 8:
                        m = i
                        pkc = ps.tile([128, L2], FP32, tag="pB", bufs=2, name=f"pkc{m}")
                        for kp in range(3):
                            nc.tensor.matmul(pkc,
                                             w_kv[:, kp, :, m * 128:(m + 1) * 128],
                                             a_t[:, kp, :, :],
                                             start=(kp == 0), stop=(kp == 2),
                                             perf_mode=DR)
                        nc.vector.tensor_copy(kcT[m], pkc)
                    else:
                        t = i - 8
                        nc.vector.memset(vcb[t][:, :, D:D + 1], 1.0)
                        for g in range(2):
                            pvc = ps.tile([128, 512], FP32, tag="pB", bufs=2,
                                          name=f"pvc{t}_{g}")
                            for kp in range(3):
                                nc.tensor.matmul(
                                    pvc, a_t[:, kp, :, t * 128:(t + 1) * 128],
                                    w_kv[:, kp, :, C + g * 512:C + (g + 1) * 512],
                                    start=(kp == 0), stop=(kp == 2),
                                    perf_mode=DR)
                            nc.vector.tensor_copy(
                                vcb[t][:, g * 8:(g + 1) * 8, 0:D],
                                pvc.rearrange("p (h d) -> p h d", h=8))

                def adaB_piece(i):
                    # modsT cols 16+4i .. 20+4i
                    j0 = 16 + 4 * i
                    pmB = ps.tile([128, 4], FP32, tag="pB", bufs=2, name=f"pmB{i}")
                    for jj in range(4):
                        for k in range(CT):
                            nc.tensor.matmul(pmB[:, jj:jj + 1],
                                             wadaB_t[i][:, k, jj * 128:(jj + 1) * 128],
                                             silu_bf[:, k:k + 1],
                                             start=(k == 0), stop=(k == CT - 1))
                    nc.vector.tensor_add(modsT[:, j0:j0 + 4], pmB,
                                         c_adab[:, j0:j0 + 4])
                    if i == 5:
                        nc.vector.tensor_scalar(out=w3eff, in0=modsT[:, 32:40],
                                                scalar1=1.0, scalar2=None,
                                                op0=mybir.AluOpType.add)
                        nc.vector.tensor_mul(w3eff, w3eff, c_n3)

                mark("self_attn")
                # =========== self-attention (software-pipelined stream) ===========
                # stream of (h, t) items; po lags LAG items behind its exp so
                # PE never blocks on Act, and head boundaries overlap.
                LAG = 3
                pos = {}
                pexps = {}
                pending = []  # (due_item, closure) in issue order

                def sa_epilogue(h):
                    def run():
                        m = h // 2
                        rs = slice((h % 2) * 64, (h % 2) * 64 + 64)
                        po = pos.pop(h)
                        rec = pp.tile([1, LQ], FP32, tag="rec", bufs=2,
                                      name=f"rec{h}")
                        nc.vector.reciprocal(rec, po[64:65, :])
                        rec_bf = pp.tile([1, LQ], BF16, tag="rec_bf", bufs=2,
                                         name=f"recb{h}")
                        nc.vector.tensor_copy(rec_bf, rec)
                        pbc = ps.tile([64, LQ], FP32, tag="pA", bufs=4,
                                      name=f"pbc{h}")
                        nc.tensor.matmul(pbc, ones_row[:, 0:64], rec_bf,
                                         start=True, stop=True)
                        rb_sb = pp.tile([64, LQ], BF16, tag="rb_sb", bufs=2,
                                        name=f"rb{h}")
                        nc.vector.tensor_copy(rb_sb, pbc)
                        nc.vector.tensor_mul(att2[m // 2][rs, m % 2, :],
                                             po[0:64, :], rb_sb)
                    return run

                def sa_po(h, t):
                    def run():
                        nc.tensor.matmul(pos[h], vsb[t][:, h, :], pexps.pop((h, t)),
                                         start=(t == 0), stop=(t == LKT - 1))
                    return run

                NIT = H * LKT
                for g in range(NIT + LKT):
                    while pending and pending[0][0] <= g:
                        pending.pop(0)[1]()
                    if g >= NIT:
                        continue
                    h, t = divmod(g, LKT)
                    m = h // 2
                    rs = slice((h % 2) * 64, (h % 2) * 64 + 64)
                    if t == 0:
                        pos[h] = ps.tile([65, LQ], FP32, tag="pC", bufs=2,
                                         name=f"po{h}")
                        if h < 8:
                            adaB_fetch(h)
                    psc = ps.tile([128, LQ], FP32, tag="pA", bufs=4,
                                  name=f"psc{h}_{t}")
                    nc.tensor.matmul(psc, kT[m][rs, t * 128:(t + 1) * 128],
                                     qT[m][rs, :], start=True, stop=True)
                    pexp = pp.tile([128, LQ], BF16, tag="pexp", bufs=6,
                                   name=f"pexp{h}_{t}")
                    nc.scalar.activation(out=pexp, in_=psc,
                                         func=mybir.ActivationFunctionType.Exp,
                                         scale=KSC)
                    pexps[(h, t)] = pexp
                    pending.append((g + LAG, sa_po(h, t)))
                    if t == LKT - 1:
                        pending.append((g + LAG + 3, sa_epilogue(h)))
                        if 2 <= h < 14:
                            pending.append((g + LAG + 5, (lambda hh:
                                lambda: cross_kv_piece(hh - 2))(h)))
                        if h >= 8:
                            pending.append((g + LAG + 7, (lambda hh:
                                lambda: adaB_piece(hh - 8))(h)))
                while pending:
                    pending.pop(0)[1]()

                mark("sa_out")
                # =========== self-attn out proj + gated residual ===========
                for m in range(CT):
                    pso = ps.tile([128, LQ], FP32, tag="pA", bufs=4, name=f"pso{m}")
                    proj_dr(pso, w_sa, m, lambda kp: att2[kp][:, :, :])
                    nc.vector.scalar_tensor_tensor(
                        out=xres[:, m, :], in0=pso, scalar=g_sa(m), in1=xres[:, m, :],
                        op0=mybir.AluOpType.mult, op1=mybir.AluOpType.add)
                pat_cm.__exit__(None, None, None)

            mark("cross")
            # =========== cross attention + MLP ===========
            with tc.tile_pool(name="pca", bufs=1) as pca:
                w_qc = pca.tile([128, KP, 2, C], F8, tag="w_qc")
                dma(out=w_qc, in_=wqc2[:, :, :, :])
                w_ca = pca.tile([128, KP, 2, C], F8, tag="w_ca")
                dma(out=w_ca, in_=wca2[:, :, :, :])
                # MLP gate/up weight stream (2 mg ahead)
                wgh_t, wgl_t, wuh_t, wul_t = {}, {}, {}, {}

                def gu_fetch(mg):
                    for d, src_, nm in ((wgh_t, wgh, "gh"), (wgl_t, wgl, "gl"),
                                        (wuh_t, wuh, "uh"), (wul_t, wul, "ul")):
                        d[mg] = pca.tile([128, KP, 2, 512], F8, tag="wgu", bufs=8,
                                         name=f"w{nm}{mg}")
                        dma(out=d[mg], in_=src_[mg])

                gu_fetch(0)
                gu_fetch(1)

                # norm2 (no modulation) -> xnb2 fp8 DR-paired
                pssq = ps.tile([1, LQ], FP32, tag="pB", bufs=2, name="pssq_n2")
                xnb2 = [pca.tile([128, 2, LQ], F8, tag=f"xn{j}", name=f"xnb{j}")
                        for j in range(KP)]
                for k in range(CT):
                    xsq = pca.tile([128, LQ], BF16, tag="xsq2", bufs=1, name=f"xsq2_{k}")
                    nc.vector.tensor_mul(xsq, xres[:, k, :], xres[:, k, :])
                    nc.tensor.matmul(pssq, ones_col, xsq,
                                     start=(k == 0), stop=(k == CT - 1))
                rstd = pca.tile([1, LQ], FP32, tag="rstd", bufs=1, name="rstd_n2")
                nc.scalar.activation(out=rstd, in_=pssq,
                                     func=mybir.ActivationFunctionType.Sqrt,
                                     bias=eps_c, scale=1.0 / C)
                nc.vector.reciprocal(rstd, rstd)
                rstd_bf = pca.tile([1, LQ], BF16, tag="rstd_bf", bufs=1, name="rstdb_n2")
                nc.vector.tensor_copy(rstd_bf, rstd)
                pb2 = ps.tile([128, LQ], FP32, tag="pA", bufs=4, name="pb_n2")
                nc.tensor.matmul(pb2, ones_row, rstd_bf, start=True, stop=True)
                for k in range(CT):
                    nc.vector.scalar_tensor_tensor(
                        out=xnb2[k // 2][:, k % 2, :], in0=xres[:, k, :],
                        scalar=c_n2[:, k:k + 1], in1=pb2,
                        op0=mybir.AluOpType.mult, op1=mybir.AluOpType.mult)

                def proj_dr2(out_psum, w, m, xcols, nkp=KP):
                    for kp in range(nkp):
                        nc.tensor.matmul(out_psum,
                                         w[:, kp, :, m * 128:(m + 1) * 128],
                                         xcols(kp),
                                         start=(kp == 0), stop=(kp == nkp - 1),
                                         perf_mode=DR)

                # cross q projection
                qcT = [pca.tile([128, LQ], BF16, tag=f"qc{m}", name=f"qcT{m}")
                       for m in range(CT)]
                for m in range(CT):
                    pq = ps.tile([128, LQ], FP32, tag="pA", bufs=4, name=f"pqc{m}")
                    proj_dr2(pq, w_qc, m, lambda kp: xnb2[kp][:, :, :])
                    nc.scalar.copy(out=qcT[m], in_=pq)

                mark("cross_attn")
                # attention over audio (software-pipelined stream)
                CLAG = 2
                pos = {}
                pexps = {}
                pending = []

                def ca_epilogue(h):
                    def run():
                        m = h // 2
                        rs = slice((h % 2) * 64, (h % 2) * 64 + 64)
                        po = pos.pop(h)
                        rec = pp.tile([1, LQ], FP32, tag="rec", bufs=2,
                                      name=f"recc{h}")
                        nc.vector.reciprocal(rec, po[64:65, :])
                        rec_bf = pp.tile([1, LQ], BF16, tag="rec_bf", bufs=2,
                                         name=f"recbc{h}")
                        nc.vector.tensor_copy(rec_bf, rec)
                        pbc = ps.tile([64, LQ], FP32, tag="pA", bufs=4,
                                      name=f"pbcc{h}")
                        nc.tensor.matmul(pbc, ones_row[:, 0:64], rec_bf,
                                         start=True, stop=True)
                        rb_sb = pp.tile([64, LQ], BF16, tag="rb_sb", bufs=2,
                                        name=f"rbc{h}")
                        nc.vector.tensor_copy(rb_sb, pbc)
                        nc.vector.tensor_mul(att2[m // 2][rs, m % 2, :],
                                             po[0:64, :], rb_sb)
                    return run

                def ca_po(h, t):
                    def run():
                        nc.tensor.matmul(pos[h], vcb[t][:, h, :], pexps.pop((h, t)),
                                         start=(t == 0), stop=(t == 3))
                    return run

                NIT = H * 4
                for g in range(NIT + 8):
                    while pending and pending[0][0] <= g:
                        pending.pop(0)[1]()
                    if g >= NIT:
                        continue
                    h, t = divmod(g, 4)
                    m = h // 2
                    rs = slice((h % 2) * 64, (h % 2) * 64 + 64)
                    if t == 0:
                        pos[h] = ps.tile([65, LQ], FP32, tag="pC", bufs=2,
                                         name=f"poc{h}")
                    psc = ps.tile([128, LQ], FP32, tag="pA", bufs=4,
                                  name=f"pscc{h}_{t}")
                    nc.tensor.matmul(psc, kcT[m][rs, t * 128:(t + 1) * 128],
                                     qcT[m][rs, :], start=True, stop=True)
                    pexp = pp.tile([128, LQ], BF16, tag="pexp", bufs=6,
                                   name=f"pexpc{h}_{t}")
                    nc.scalar.activation(out=pexp, in_=psc,
                                         func=mybir.ActivationFunctionType.Exp,
                                         scale=KSC)
                    pexps[(h, t)] = pexp
                    pending.append((g + CLAG, ca_po(h, t)))
                    if t == 3:
                        pending.append((g + CLAG + 2, ca_epilogue(h)))
                        if h % 3 == 0 and 2 + h // 3 < 8:
                            pending.append((g + CLAG + 3, (lambda mg:
                                lambda: gu_fetch(mg))(2 + h // 3)))
                while pending:
                    pending.pop(0)[1]()

                mark("ca_out")
                # cross out proj + residual (no gate)
                for m in range(CT):
                    pco = ps.tile([128, LQ], FP32, tag="pA", bufs=4, name=f"pcao{m}")
                    proj_dr2(pco, w_ca, m, lambda kp: att2[kp][:, :, :])
                    nc.vector.tensor_add(xres[:, m, :], xres[:, m, :], pco)

                mark("mlp_norm")
                # norm3 + modulation -> bf16, then hi/lo fp8 split
                pssq3 = ps.tile([1, LQ], FP32, tag="pB", bufs=2, name="pssq_n3")
                xmb = [pca.tile([128, LQ], BF16, tag=f"xm{k}", name=f"xmb{k}")
                       for k in range(CT)]
                xh2 = [pca.tile([128, 2, LQ], F8, tag=f"xh{j}", name=f"xh{j}")
                       for j in range(KP)]
                xl2 = [pca.tile([128, 2, LQ], F8, tag=f"xl{j}", name=f"xl{j}")
                       for j in range(KP)]
                x64 = [pca.tile([128, 2, LQ], F8, tag=f"x6{j}", name=f"x6{j}")
                       for j in range(KP)]
                for k in range(CT):
                    xsq = pca.tile([128, LQ], BF16, tag="xsq2", bufs=1, name=f"xsq3_{k}")
                    nc.vector.tensor_mul(xsq, xres[:, k, :], xres[:, k, :])
                    nc.tensor.matmul(pssq3, ones_col, xsq,
                                     start=(k == 0), stop=(k == CT - 1))
                rstd3 = pca.tile([1, LQ], FP32, tag="rstd", bufs=1, name="rstd_n3")
                nc.scalar.activation(out=rstd3, in_=pssq3,
                                     func=mybir.ActivationFunctionType.Sqrt,
                                     bias=eps_c, scale=1.0 / C)
                nc.vector.reciprocal(rstd3, rstd3)
                rstd3_bf = pca.tile([1, LQ], BF16, tag="rstd_bf", bufs=1,
                                    name="rstdb_n3")
                nc.vector.tensor_copy(rstd3_bf, rstd3)
                pb3 = ps.tile([128, LQ], FP32, tag="pA", bufs=4, name="pb_n3")
                nc.tensor.matmul(pb3, ones_row, rstd3_bf, start=True, stop=True)
                for k in range(CT):
                    nc.vector.scalar_tensor_tensor(
                        out=xmb[k], in0=xres[:, k, :], scalar=w3eff[:, k:k + 1],
                        in1=pb3,
                        op0=mybir.AluOpType.mult, op1=mybir.AluOpType.mult)
                    nc.gpsimd.tensor_scalar(out=xmb[k], in0=xmb[k],
                                            scalar1=sh_ml(k), scalar2=None,
                                            op0=mybir.AluOpType.add)
                    hi = xh2[k // 2][:, k % 2, :]
                    lo = xl2[k // 2][:, k % 2, :]
                    nc.scalar.copy(out=hi, in_=xmb[k])
                    nc.vector.tensor_sub(lo, xmb[k], hi)
                    nc.scalar.activation(out=x64[k // 2][:, k % 2, :], in_=xmb[k],
                                         func=mybir.ActivationFunctionType.Identity,
                                         scale=1.0 / LOSC)

                mark("gate_up")
                # h2: fp8 DR-paired ffn activations
                h2 = [pca.tile([128, 2, LQ], F8, tag=f"h{t}", name=f"h2_{t}")
                      for t in range(FF // 256)]
                h64_2 = [pca.tile([128, 2, LQ], F8, tag=f"h6{t}", name=f"h64_{t}")
                         for t in range(FF // 256)]
                wdh_t, wdl_t = {}, {}

                def down_fetch(m):
                    wdh_t[m] = pca.tile([128, 16, 2, 128], F8, tag="wdw", bufs=4,
                                        name=f"wdh{m}")
                    dma(out=wdh_t[m], in_=wdh[m])
                    wdl_t[m] = pca.tile([128, 16, 2, 128], F8, tag="wdw", bufs=4,
                                        name=f"wdl{m}")
                    dma(out=wdl_t[m], in_=wdl[m])

                def dr_hilo(p1, wh, wl, mi):
                    # Xh*Wh + (X/64)*(Wl*64) + Xl*Wh, all at true scale
                    ms = slice(mi * 128, (mi + 1) * 128)
                    for kp in range(KP):
                        nc.tensor.matmul(p1, wh[:, kp, :, ms], xh2[kp][:, :, :],
                                         start=(kp == 0), stop=False, perf_mode=DR)
                    for kp in range(KP):
                        nc.tensor.matmul(p1, wl[:, kp, :, ms], x64[kp][:, :, :],
                                         start=False, stop=False, perf_mode=DR)
                    for kp in range(KP):
                        nc.tensor.matmul(p1, wh[:, kp, :, ms], xl2[kp][:, :, :],
                                         start=False, stop=(kp == KP - 1),
                                         perf_mode=DR)

                for mg in range(8):
                    if mg >= 6:
                        down_fetch(mg - 6)
                    for mi in range(4):
                        p1g = ps.tile([128, LQ], FP32, tag="pA", bufs=4,
                                      name=f"p1g{mg}_{mi}")
                        dr_hilo(p1g, wgh_t[mg], wgl_t[mg], mi)
                        sg = pca.tile([128, LQ], BF16, tag="sgb", bufs=2,
                                      name=f"sg{mg}_{mi}")
                        nc.scalar.activation(out=sg, in_=p1g,
                                             func=mybir.ActivationFunctionType.Sigmoid)
                        gbf = pca.tile([128, LQ], BF16, tag="gbf", bufs=4,
                                       name=f"gbf{mg}_{mi}")
                        nc.vector.tensor_mul(gbf, sg, p1g)
                        p1u = ps.tile([128, LQ], FP32, tag="pA", bufs=4,
                                      name=f"p1u{mg}_{mi}")
                        dr_hilo(p1u, wuh_t[mg], wul_t[mg], mi)
                        t = mg * 4 + mi
                        nc.vector.tensor_mul(h2[t // 2][:, t % 2, :], gbf, p1u)
                        h64 = h64_2[t // 2][:, t % 2, :]
                        nc.scalar.activation(
                            out=h64, in_=h2[t // 2][:, t % 2, :],
                            func=mybir.ActivationFunctionType.Identity,
                            scale=1.0 / LOSC)

                mark("down")
                # down proj: P1 = H*Wdh, P2 = H*Wdl(x64); out = (P1 + P2/64)*g + xres
                for m in range(CT):
                    if m + 2 < CT:
                        down_fetch(m + 2)
                    pd1 = ps.tile([128, LQ], FP32, tag="pA", bufs=4, name=f"pd1{m}")
                    for fp in range(16):
                        nc.tensor.matmul(pd1, wdh_t[m][:, fp, :, :],
                                         h2[fp][:, :, :],
                                         start=(fp == 0), stop=False,
                                         perf_mode=DR)
                    for fp in range(16):
                        nc.tensor.matmul(pd1, wdl_t[m][:, fp, :, :],
                                         h64_2[fp][:, :, :],
                                         start=False, stop=(fp == 15),
                                         perf_mode=DR)
                    of = pca.tile([128, LQ], FP32, tag="of", bufs=2, name=f"of{m}")
                    nc.vector.scalar_tensor_tensor(
                        out=of, in0=pd1, scalar=g_ml(m), in1=xres[:, m, :],
                        op0=mybir.AluOpType.mult, op1=mybir.AluOpType.add)
                    dma(out=outT[m * 128:(m + 1) * 128, :], in_=of)

    nc.compile()
    return nc


_ROPE_PERM = None
_SWAP_PERM = None


def _perms():
    global _ROPE_PERM, _SWAP_PERM
    if _ROPE_PERM is None:
        p = np.zeros(C, dtype=np.int64)
        s = np.zeros(C, dtype=np.int64)
        for h in range(H):
            for i in range(D // 2):
                p[h * D + i] = h * D + 2 * i               # real block
                p[h * D + D // 2 + i] = h * D + 2 * i + 1  # imag block
                s[h * D + i] = h * D + 2 * i + 1           # swapped: imag first
                s[h * D + D // 2 + i] = h * D + 2 * i
        _ROPE_PERM, _SWAP_PERM = p, s
    return _ROPE_PERM, _SWAP_PERM


def _bf(a):
    return np.ascontiguousarray(a).astype(ml_dtypes.bfloat16)


def _f8(a):
    return np.ascontiguousarray(a).astype(F8NP)


def _dr_pack(W):
    # [n_in, n_out] -> [128, n_in//256, 2, n_out]
    n_in, n_out = W.shape
    kp = n_in // 256
    return W.reshape(kp, 2, 128, n_out).transpose(2, 0, 1, 3)


def _hilo(W):
    hi = W.astype(F8NP)
    lo = ((W - hi.astype(np.float32)) * LOSC).astype(F8NP)
    return hi, lo


def _prep_shared(W_qkv, W_sa_out, W_q, W_kv, W_ca_out, W_gate, W_up, W_down,
                 adaLN_W, adaLN_b, norm1_w, norm2_w, norm3_w):
    perm, sperm = _perms()
    wq = W_qkv[:, 0:C][:, perm]
    wqs = W_qkv[:, 0:C][:, sperm]
    wk = W_qkv[:, C:2 * C][:, perm]
    wks = W_qkv[:, C:2 * C][:, sperm]
    wv = W_qkv[:, 2 * C:3 * C]

    def pack8(W):
        return _f8(_dr_pack(np.asarray(W, np.float32)))

    wgh_, wgl_ = _hilo(np.asarray(W_gate, np.float32))
    wuh_, wul_ = _hilo(np.asarray(W_up, np.float32))
    wdh_, wdl_ = _hilo(np.asarray(W_down, np.float32))

    def mlp_pack(w8):  # fp8 [C, FF] -> [8 mg][128, kp, 2, 512]
        d = _dr_pack(w8.astype(np.float32)).astype(F8NP)  # [128, 4, 2, 4096]
        return np.ascontiguousarray(d.reshape(128, KP, 2, 8, 512)
                                    .transpose(3, 0, 1, 2, 4))

    def down_pack(w8):  # fp8 [FF, C] -> [8 m][128, 16 fp, 2, 128]
        d = _dr_pack(w8.astype(np.float32)).astype(F8NP)  # [128, 16, 2, C]
        return np.ascontiguousarray(d.reshape(128, 16, 2, CT, 128)
                                    .transpose(3, 0, 1, 2, 4))

    # adaLN weight-stationary tiles: [p, k, j*128+q] = W[128k+p, 128j+q]
    wada = np.asarray(adaLN_W, np.float32).reshape(CT, 128, 48, 128)
    wadaA_h = wada[:, :, 0:16, :].transpose(1, 0, 2, 3).reshape(128, CT, 2048)
    wadaB_h = np.stack([
        wada[:, :, 16 + 4 * i:20 + 4 * i, :].transpose(1, 0, 2, 3)
        .reshape(128, CT, 512) for i in range(8)])

    sh = {
        "wq2": pack8(wq), "wqs2": pack8(wqs), "wk2": pack8(wk),
        "wks2": pack8(wks), "wv2": pack8(wv),
        "wsa2": pack8(W_sa_out), "wqc2": pack8(W_q), "wkv2": pack8(W_kv),
        "wca2": pack8(W_ca_out),
        "wgh": mlp_pack(wgh_), "wgl": mlp_pack(wgl_),
        "wuh": mlp_pack(wuh_), "wul": mlp_pack(wul_),
        "wdh": down_pack(wdh_), "wdl": down_pack(wdl_),
        "wadaA": _bf(wadaA_h), "wadaB": _bf(wadaB_h),
        "adabT": np.ascontiguousarray(
            np.asarray(adaLN_b, np.float32).reshape(48, 128).T),
        "n1w": np.ascontiguousarray(
            np.asarray(norm1_w, np.float32).reshape(8, 128).T),
        "n2w": np.ascontiguousarray(
            np.asarray(norm2_w, np.float32).reshape(8, 128).T),
        "n3w": np.ascontiguousarray(
            np.asarray(norm3_w, np.float32).reshape(8, 128).T),
    }
    return sh


def make_in_maps(x, t_mod, audio_context, freqs_cos, freqs_sin,
                 norm1_w, norm2_w, norm3_w,
                 W_qkv, W_sa_out, W_q, W_kv, W_ca_out,
                 W_gate, W_up, W_down, adaLN_W, adaLN_b):
    sh = _prep_shared(W_qkv, W_sa_out, W_q, W_kv, W_ca_out, W_gate, W_up,
                      W_down, adaLN_W, adaLN_b, norm1_w, norm2_w, norm3_w)
    cosT = np.ascontiguousarray(np.asarray(freqs_cos, np.float32).T)
    sinT = np.ascontiguousarray(np.asarray(freqs_sin, np.float32).T)

    in_maps = []
    for core in range(NCORE):
        b, j = divmod(core, 4)
        # roll the token axis so this core's own 512 tokens sit at [0, LQ)
        xT = np.roll(np.ascontiguousarray(np.asarray(x, np.float32)[b].T),
                     -j * LQ, axis=1)
        m = dict(sh)
        m["x_bf"] = _bf(xT)
        m["xq_f"] = np.ascontiguousarray(xT[:, 0:LQ])
        cr = np.roll(cosT, -j * LQ, axis=1)
        sr = np.roll(sinT, -j * LQ, axis=1)
        m["cs4"] = _bf(np.concatenate([cr, cr, cr, cr], axis=0))
        m["sc4"] = _bf(np.concatenate([-sr, sr, -sr, sr], axis=0))
        m["aud2"] = _f8(_dr_pack(
            np.ascontiguousarray(np.asarray(audio_context, np.float32)[b].T)))
        m["tmodT"] = np.ascontiguousarray(
            np.asarray(t_mod, np.float32)[b].reshape(8, 128).T)
        in_maps.append(m)
    return in_maps


_NC_CACHE = None


def _get_nc():
    global _NC_CACHE
    if _NC_CACHE is None:
        _NC_CACHE = build_bass()
    return _NC_CACHE


def kernel(**inputs):
    nc = _get_nc()
    inputs = {k: np.asarray(v) for k, v in inputs.items()}
    in_maps = make_in_maps(**inputs)
    res = run_bass_kernel_spmd(nc, in_maps, list(range(NCORE)))
    out = np.zeros((B, L, C), np.float32)
    for core in range(NCORE):
        b, j = divmod(core, 4)
        out[b, j * LQ:(j + 1) * LQ, :] = res.results[core]["outT"].T
    return out
